# revision 1
# baseline (speedup 1.0000x reference)
"""Multi-head attention (softmax over the QUERY axis) for Trainium2, 8 cores.

Reference computation (B=2, T=2048, E=1024, H=16, HD=64):
    q = split_heads(X @ Wq.T + bq); k = ...; v = ...
    scores = (q @ k^T) / sqrt(E), causally masked (key > query -> -inf)
    attn   = softmax(scores, axis=QUERY)      # <- normalizes over q, per key
    out    = attn @ v, merged heads

Sharding: core c handles batch c//4 and head group c%4 (4 heads = 256 output
dims).  No collectives needed.  Host pre-transposes X and the weight slices so
the device never transposes anything.

Device layout (per core).  Projection/scores matmul operands are declared
float32r (full PE rate, vs 4 cycles/row for plain fp32); the A*V operands
(P, V') are float16, which permits col tile_position packing (fp32r does
not pass the walrus ISA check for it):
    XT [1024,2048]  (e on partitions via 8 chunks of 128)
    QT/KT = W.T.T @ XT + b  ->  [256 d, 2048 t]   (2 partition tiles of 128)
    V     = XT.T @ WvT + bv ->  [2048 t, 256 d]   (16 t-tiles of 128)
    per (duo of 2 heads, k-tile of 128 keys):
        S^T[k, q] = K^T.T @ Q^T  (heads packed in the PE via row tile_position)
        mask diag block, P = exp(S^T/32) via ACT (PSUM->SBUF) + row-sum accum
        r = 1/sum, V' = V * r (per-key scaling replaces the softmax divide)
        O^T[d, q] += V'.T @ P   (fp16; the two heads' matmuls run in separate
        PE column groups concurrently, accumulating over k-tiles into four
        per-bank PSUM tiles; bank 0 is time-shared with projections/V)
    O^T [256, 2048] -> DRAM; host writes out[b, :, g*256:(g+1)*256] = O^T.T
"""

from contextlib import ExitStack

import numpy as np

import concourse.bacc as bacc
import concourse.mybir as mybir
import concourse.tile as tile
from concourse.bass_utils import run_bass_kernel_spmd

B, T, E, H = 2, 2048, 1024, 16
HD = 64
D2 = 256           # output dims per core (4 heads)
NKT = T // 128     # 16 k-tiles
F32 = mybir.dt.float32
F32R = mybir.dt.float32r
F16 = mybir.dt.float16
MDT = F32R     # dtype for all matmul operands (PE runs full speed on f32r)
EXP = mybir.ActivationFunctionType.Exp
AX = mybir.AxisListType.X
SCALE = 1.0 / 32.0  # 1/sqrt(E)
NEG = -1.0e30

_CACHE = {}


def _build_module():
    nc = bacc.Bacc("TRN2", target_bir_lowering=False, debug=False)

    xt_d = nc.dram_tensor("xt", [E, T], MDT, kind="ExternalInput")
    wqt_d = nc.dram_tensor("wqt", [E, D2], MDT, kind="ExternalInput")
    wkt_d = nc.dram_tensor("wkt", [E, D2], MDT, kind="ExternalInput")
    wvt_d = nc.dram_tensor("wvt", [E, D2], MDT, kind="ExternalInput")
    bqc_d = nc.dram_tensor("bqc", [128, 2], F32, kind="ExternalInput")
    bkc_d = nc.dram_tensor("bkc", [128, 2], F32, kind="ExternalInput")
    bvr_d = nc.dram_tensor("bvr", [1, D2], MDT, kind="ExternalInput")
    mask_d = nc.dram_tensor("mask", [128, 128], F32, kind="ExternalInput")
    ones_d = nc.dram_tensor("onesr", [1, 512], MDT, kind="ExternalInput")
    zr_d = nc.dram_tensor("zr", [128, 128], MDT, kind="ExternalInput")
    ot_d = nc.dram_tensor("ot", [D2, T], F32, kind="ExternalOutput")

    with tile.TileContext(nc) as tc:
        _body(tc, xt_d, wqt_d, wkt_d, wvt_d, bqc_d, bkc_d, bvr_d, mask_d,
              ones_d, zr_d, ot_d)
    nc.compile()
    return nc


def _body(tc, xt_d, wqt_d, wkt_d, wvt_d, bqc_d, bkc_d, bvr_d, mask_d,
          ones_d, zr_d, ot_d):
    nc = tc.nc

    with ExitStack() as ctx:
        const_pool = ctx.enter_context(tc.tile_pool(name="const", bufs=1))
        ones_t = const_pool.tile([1, 512], MDT)
        nc.sync.dma_start(ones_t[:], ones_d.ap())
        mask_t = const_pool.tile([128, 128], F32)
        nc.sync.dma_start(mask_t[:], mask_d.ap())
        bqc_t = const_pool.tile([128, 2], F32)
        nc.sync.dma_start(bqc_t[:], bqc_d.ap())
        bkc_t = const_pool.tile([128, 2], F32)
        nc.sync.dma_start(bkc_t[:], bkc_d.ap())
        bvr_t = const_pool.tile([1, D2], MDT)
        nc.sync.dma_start(bvr_t[:], bvr_d.ap())

        # V'-weights ping-pong tiles, fp16: cols [0:64]=vp_h0,
        # [64:128]=vp_h1.  The A*V matmuls run in fp16 so the two heads pack
        # into the PE concurrently via col tile_position (fp32r col-packing
        # fails walrus ISA checks; fp16 is the documented packing path).
        vp_pool = ctx.enter_context(tc.tile_pool(name="vp", bufs=1))
        vp_ab = []
        for i in range(2):
            vp = vp_pool.tile([128, 128], F16, name=f"vp{i}")
            vp_ab.append(vp)

        proj_pool = ctx.enter_context(tc.tile_pool(name="proj", bufs=1))
        qt_t = proj_pool.tile([128, 2 * T], MDT)   # [:, dt*T + t]
        kt_t = proj_pool.tile([128, 2 * T], MDT)
        v_t = proj_pool.tile([128, NKT * D2], F32)  # [:, tt*D2 + d]

        # Projections are issued on demand inside the attention loop, so duo
        # 1's projections fill PE gaps while duo 0's attention keeps ACT busy.
        # PSUM budget (8 banks): 3x [128,512] scores slots (exp pipeline
        # stays fed), 1x [128,512] projection/V slot, 4-bank O^T accumulator.
        with (
            tc.tile_pool(name="xt", bufs=1) as xt_pool,
            tc.tile_pool(name="w", bufs=1) as w_pool,
            tc.tile_pool(name="p", bufs=2) as p_pool,
            tc.tile_pool(name="stats", bufs=3) as st_pool,
            tc.tile_pool(name="osb", bufs=1) as osb_pool,
            tc.tile_pool(name="sc_ps", bufs=2, space="PSUM") as sc_pool,
            tc.tile_pool(name="ot_ps", bufs=1, space="PSUM") as ot_pool,
        ):
            # Warm the ACT exp table off the critical path (first real exp
            # otherwise pays the ~1.3us table load mid-pipeline).
            warm_t = st_pool.tile([1, 2], F32, name="warm")
            nc.scalar.activation(warm_t[:], mask_t[0:1, 0:2], EXP,
                                 bias=0.0, scale=SCALE)

            # DMA order: wq/wk chunk just before its xt chunk (so Q/K
            # projections complete right as the last xt chunk lands); wv is
            # only needed for V tiles, which trail — load it last.
            xt_t = xt_pool.tile([128, 8 * T], MDT)  # [:, ec*T + t]
            wq_t = w_pool.tile([128, 8 * D2], MDT)  # [:, ec*D2 + d]
            wk_t = w_pool.tile([128, 8 * D2], MDT)
            wv_t = w_pool.tile([128, 8 * D2], MDT)
            for ec in range(8):
                for w_sb, w_dr in ((wq_t, wqt_d), (wk_t, wkt_d)):
                    nc.sync.dma_start(
                        w_sb[:, ec * D2:(ec + 1) * D2],
                        w_dr.ap()[ec * 128:(ec + 1) * 128, :],
                    )
                nc.sync.dma_start(
                    xt_t[:, ec * T:(ec + 1) * T],
                    xt_d.ap()[ec * 128:(ec + 1) * 128, :],
                )
            for ec in range(8):
                nc.sync.dma_start(
                    wv_t[:, ec * D2:(ec + 1) * D2],
                    wvt_d.ap()[ec * 128:(ec + 1) * 128, :],
                )

            def emit_v_tile(tt):
                # V[tt]: [128 t, D2] = XT.T @ WvT + ones.T @ bv
                ps = ot_pool.tile([128, 512], F32, tag="ot0", name="ps_v")
                pv = ps[:, 0:D2]
                for ec in range(8):
                    nc.tensor.matmul(
                        pv,
                        lhsT=xt_t[:, ec * T + tt * 128:ec * T + tt * 128 + 128],
                        rhs=wv_t[:, ec * D2:(ec + 1) * D2],
                        start=(ec == 0),
                        stop=False,
                    )
                nc.tensor.matmul(
                    pv,
                    lhsT=ones_t[0:1, 0:128],
                    rhs=bvr_t[0:1, :],
                    start=False,
                    stop=True,
                )
                nc.vector.tensor_copy(v_t[:, tt * D2:(tt + 1) * D2], pv)

            def emit_qk_chunk(pduo, is_k, c, on_ot0=False):
                # one 512-wide QT/KT projection chunk for duo `pduo`.
                # Chunks that run while O^T bank 0 is idle (duo 1's, injected
                # into duo 0's late k-tiles; startup before the accumulator
                # exists) time-share the ot0 bank; chunks needed while ot0 is
                # live briefly borrow a scores slot instead.
                out_t, w_sb, b_sb = ((kt_t, wk_t, bkc_t) if is_k
                                     else (qt_t, wq_t, bqc_t))
                if on_ot0:
                    ps = ot_pool.tile([128, 512], F32, tag="ot0", name="ps_qk")
                else:
                    ps = sc_pool.tile([128, 512], F32, tag="sc", name="ps_qk")
                for ec in range(8):
                    nc.tensor.matmul(
                        ps[:],
                        lhsT=w_sb[:, ec * D2 + pduo * 128:
                                  ec * D2 + pduo * 128 + 128],
                        rhs=xt_t[:, ec * T + c * 512:ec * T + c * 512 + 512],
                        start=(ec == 0),
                        stop=(ec == 7),
                    )
                nc.vector.tensor_scalar_add(
                    out_t[:, pduo * T + c * 512:pduo * T + c * 512 + 512],
                    ps[:],
                    b_sb[:, pduo:pduo + 1],
                )

            emitted = set()

            def ensure_qk(pduo, is_k, c, on_ot0=False):
                if (pduo, is_k, c) not in emitted:
                    emitted.add((pduo, is_k, c))
                    emit_qk_chunk(pduo, is_k, c, on_ot0=on_ot0)

            for duo in range(2):
                # Projections are emitted on demand (first use by a scores
                # piece), so the exp pipeline starts as early as possible.
                # kt0's chunks are pre-emitted spread over BOTH psum pools so
                # they accumulate concurrently while X streams in; duo 1's
                # chunks are injected into duo 0's late k-tiles so they fill
                # PE slack while ACT stays busy.
                if duo == 0:
                    ensure_qk(0, False, 0)
                    ensure_qk(0, False, 1)
                    ensure_qk(0, True, 0, on_ot0=True)
                    ensure_qk(0, False, 2)
                    ensure_qk(0, False, 3)
                    for tt in range(4):
                        emit_v_tile(tt)
                    inject = {1: [(0, True, 1)],
                              4: [(0, True, 2)],
                              6: [(1, False, 0)],
                              7: [(1, True, 0)],
                              8: [(0, True, 3)],
                              9: [(1, False, 1)],
                              10: [(1, False, 2)],
                              11: [(1, False, 3), (1, True, 1)],
                              12: [(1, True, 2)],
                              13: [(1, True, 3)]}
                else:
                    inject = {}

                # ---- attention for this duo ----
                ot_bk = [ot_pool.tile([128, 512], F32, tag=f"ot{b}",
                                      name=f"ot{b}") for b in range(4)]
                ot_sb = osb_pool.tile([128, T], F32, tag="osb", name="ot_sb")
                for kt in range(NKT):
                    qlo = kt * 128
                    W = T - qlo
                    pieces = []
                    poff = 0
                    while poff < W:
                        pieces.append((poff, min(1024, W - poff)))
                        poff += 1024

                    sums_t = st_pool.tile([128, 4], F32, tag="sums", name="sums")
                    rinv_t = st_pool.tile([128, 2], F32, tag="rinv", name="rinv")

                    p_ts = []
                    for hh in range(2):
                        p_t = p_pool.tile([128, T], F16, tag=f"p{hh}",
                                          name=f"p{hh}")
                        p_ts.append(p_t)
                        d0 = 64 * hh
                        for pi, (poff, pw) in enumerate(pieces):
                            ensure_qk(duo, True, kt // 4)
                            for c in range((qlo + poff) // 512,
                                           (qlo + poff + pw - 1) // 512 + 1):
                                ensure_qk(duo, False, c)
                            sc = sc_pool.tile([128, 1024], F32, tag="sc",
                                              name="sc")
                            for co in range(0, pw, 512):
                                n = min(512, pw - co)
                                nc.tensor.matmul(
                                    sc[:, co:co + n],
                                    lhsT=kt_t[d0:d0 + 64,
                                              duo * T + qlo:
                                              duo * T + qlo + 128],
                                    rhs=qt_t[d0:d0 + 64,
                                             duo * T + qlo + poff + co:
                                             duo * T + qlo + poff + co + n],
                                    start=True,
                                    stop=True,
                                )
                            if poff == 0:
                                nc.vector.tensor_add(sc[:, 0:128], sc[:, 0:128],
                                                     mask_t[:])
                            nc.scalar.activation(
                                p_t[:, poff:poff + pw],
                                sc[:, 0:pw],
                                EXP,
                                bias=0.0,
                                scale=SCALE,
                                accum_out=sums_t[:, hh * 2 + pi:hh * 2 + pi + 1],
                            )

                    if duo == 0 and kt < NKT - 4:
                        emit_v_tile(kt + 4)
                    for args in inject.get(kt, ()):
                        pduo_i = args[0]
                        ensure_qk(*args, on_ot0=(pduo_i == 1))

                    # 1/sum; V' = V * r into the zero-padded weight tile
                    vp_t = vp_ab[kt % 2]
                    np_ = len(pieces)
                    for hh in range(2):
                        if np_ > 1:
                            rs_t = st_pool.tile([128, 1], F32, tag=f"rs{hh}",
                                                name=f"rs{hh}")
                            nc.vector.reduce_sum(rs_t[:],
                                                 sums_t[:, hh * 2:hh * 2 + np_],
                                                 axis=AX)
                        else:
                            rs_t = sums_t[:, hh * 2:hh * 2 + 1]
                        nc.vector.reciprocal(rinv_t[:, hh:hh + 1], rs_t[:])
                        dst = vp_t[:, 0:64] if hh == 0 else vp_t[:, 64:128]
                        nc.vector.tensor_scalar_mul(
                            dst,
                            v_t[:, kt * D2 + duo * 128 + 64 * hh:
                                kt * D2 + duo * 128 + 64 * hh + 64],
                            rinv_t[:, hh:hh + 1],
                        )

                    # O^T[:, q] += V'.T @ P, bank-aligned chunks of 512.
                    # Head 0 owns start= (first write of the bank), head 1
                    # owns stop= on the bank's last k-tile; finished banks are
                    # copied out immediately so the tail stays short.
                    c0 = qlo
                    while c0 < T:
                        bank = c0 // 512
                        c1 = min((bank + 1) * 512, T)
                        last_kt = min(4 * bank + 3, NKT - 1)
                        for hh in range(2):
                            nc.tensor.matmul(
                                ot_bk[bank][64 * hh:64 * hh + 64,
                                            c0 - bank * 512:c1 - bank * 512],
                                lhsT=vp_t[:, 64 * hh:64 * hh + 64],
                                rhs=p_ts[hh][:, c0 - qlo:c1 - qlo],
                                start=(kt == 0),
                                stop=(kt == last_kt),
                            )
                        if kt == last_kt:
                            nc.vector.tensor_copy(
                                ot_sb[:, bank * 512:bank * 512 + 512],
                                ot_bk[bank][:])
                            nc.sync.dma_start(
                                ot_d.ap()[duo * 128:(duo + 1) * 128,
                                          bank * 512:bank * 512 + 512],
                                ot_sb[:, bank * 512:bank * 512 + 512])
                        c0 = c1


def _get_module():
    if "nc" not in _CACHE:
        _CACHE["nc"] = _build_module()
    return _CACHE["nc"]


def _make_mask():
    k = np.arange(128)[:, None]
    q = np.arange(128)[None, :]
    return np.where(q >= k, 0.0, NEG).astype(np.float32)


def _make_in_maps(X, Wq, bq, Wk, bk, Wv, bv):
    X = np.asarray(X, np.float32)
    mask = _make_mask()
    ones = np.ones((1, 512), np.float32)
    zr = np.zeros((128, 128), np.float32)
    in_maps = []
    for c in range(8):
        b, g = divmod(c, 4)
        rows = slice(D2 * g, D2 * g + D2)
        in_maps.append({
            "xt": np.ascontiguousarray(X[b].T),
            "wqt": np.ascontiguousarray(np.asarray(Wq)[rows].T),
            "wkt": np.ascontiguousarray(np.asarray(Wk)[rows].T),
            "wvt": np.ascontiguousarray(np.asarray(Wv)[rows].T),
            "bqc": np.ascontiguousarray(np.asarray(bq)[rows].reshape(2, 128).T),
            "bkc": np.ascontiguousarray(np.asarray(bk)[rows].reshape(2, 128).T),
            "bvr": np.ascontiguousarray(np.asarray(bv)[rows].reshape(1, D2)),
            "mask": mask,
            "onesr": ones,
            "zr": zr,
        })
    return in_maps


def kernel(X, Wq, bq, Wk, bk, Wv, bv, **kw):
    in_maps = _make_in_maps(X, Wq, bq, Wk, bk, Wv, bv)
    nc = _get_module()
    res = run_bass_kernel_spmd(nc, in_maps, core_ids=list(range(8)), **kw)
    _CACHE["last_res"] = res
    out = np.zeros((B, T, E), np.float32)
    for c in range(8):
        b, g = divmod(c, 4)
        out[b, :, D2 * g:D2 * g + D2] = res.results[c]["ot"].T
    return out


if __name__ == "__main__":
    _get_module()
    print("module built ok")



# revision 8
# speedup vs baseline: 1.3222x; 1.3222x over previous
"""Multi-head attention (softmax over the QUERY axis) for Trainium2, 8 cores.

Reference (B=2, T=2048, E=1024, H=16, HD=64):
    q = X@Wq.T+bq ; k = ... ; v = ...   (per-head split)
    s = (q k^T)/sqrt(E), causal mask (key > query -> -inf)
    attn = softmax(s, axis=QUERY)  -> normalizes each key COLUMN over queries
    out  = attn @ v

Sharding: core c = (batch c//4, head-group c%4 of 4 heads = 2 duos).  No
collectives.

Math per core (d2=256 output dims as O^T [256, T]):
  Projections run in fp8e4 DoubleRow with hi+lo error compensation:
  W' = 32*W split W8+Wl (fp8), X split X8+Xl; q32 ~= X8@W8 + Xl@W8 + X8@Wl
  (+32b).  Scores use bf16 Q/K: s_psum = 1024*s; exp applies
  scale=1/32768 and per-key bias -ln(c_k), c_k ~ sqrt(E[r_k]) (any c_k is
  mathematically exact: V-scaling self-corrects).  P~=P/c_k stored fp8
  (keys < 1792) / fp16 (last 256 keys); V~ = 32*v*c/r.  A@V runs fp8
  DoubleRow over PAIRS of key-tiles (contraction 256); final PSUM->SBUF
  copy multiplies by 1/32.

Schedule: per duo, key-tiles processed DESCENDING (small score tiles
first -> exp starts ~7us in).  All P~/V~ live in SBUF; O^T accumulation
(phase B) runs bank-at-a-time overlapped with the next duo's phase A.
PSUM: 3x[128,1024] score slots + 2x[128,512] proj/V/O slots.
"""

import math
from contextlib import ExitStack

import numpy as np
import ml_dtypes

import concourse.bacc as bacc
import concourse.mybir as mybir
import concourse.tile as tile
from concourse.bass_utils import run_bass_kernel_spmd

B, T, E, H = 2, 2048, 1024, 16
D2 = 256              # output dims per core (4 heads)
NKT = 16              # key tiles of 128
NPAIR = 7             # fp8 DoubleRow pairs: kts (2p, 2p+1), p=0..6; kts 14,15 fp16
F32 = mybir.dt.float32
BF16 = mybir.dt.bfloat16
F16 = mybir.dt.float16
F8 = mybir.dt.float8e4
DRM = mybir.MatmulPerfMode.DoubleRow
EXP = mybir.ActivationFunctionType.Exp
AX = mybir.AxisListType.X
SCALE = 1.0 / 32768.0   # 1/sqrt(E) / 32 / 32  (both Q,K carry 32x weights)
NEG = -1.0e30
WS = 32.0               # weight prescale
np_f8 = ml_dtypes.float8_e4m3
np_bf16 = ml_dtypes.bfloat16

# w_t column-block offsets (per e-chunk, 1536 wide)
Q8O, K8O, QLO, KLO, V8O, VLO = 0, 256, 512, 768, 1024, 1280

_CACHE = {}


def _build_module():
    nc = bacc.Bacc("TRN2", target_bir_lowering=False, debug=False)

    x8_d = nc.dram_tensor("x8", [128, 8, T], F8, kind="ExternalInput")
    xl_d = nc.dram_tensor("xl", [128, 8, T], F8, kind="ExternalInput")
    wqk8_d = nc.dram_tensor("wqk8", [128, 8, 512], F8, kind="ExternalInput")
    wqkl_d = nc.dram_tensor("wqkl", [128, 8, 512], F8, kind="ExternalInput")
    wv8l_d = nc.dram_tensor("wv8l", [128, 8, 512], F8, kind="ExternalInput")
    bqc_d = nc.dram_tensor("bqc", [128, 2], F32, kind="ExternalInput")
    bkc_d = nc.dram_tensor("bkc", [128, 2], F32, kind="ExternalInput")
    bvr_d = nc.dram_tensor("bvr", [1, D2], BF16, kind="ExternalInput")
    biasc_d = nc.dram_tensor("biasc", [128, NKT], F32, kind="ExternalInput")
    masku_d = nc.dram_tensor("masku", [128, 128], BF16, kind="ExternalInput")
    ident_d = nc.dram_tensor("ident", [128, 128], BF16, kind="ExternalInput")
    ones_d = nc.dram_tensor("ones", [1, 128], BF16, kind="ExternalInput")
    ot_d = nc.dram_tensor("ot", [T, D2], F32, kind="ExternalOutput")

    with tile.TileContext(nc) as tc:
        _body(tc, x8_d, xl_d, wqk8_d, wqkl_d, wv8l_d, bqc_d, bkc_d, bvr_d,
              biasc_d, masku_d, ident_d, ones_d, ot_d)
    nc.compile()
    return nc


def _body(tc, x8_d, xl_d, wqk8_d, wqkl_d, wv8l_d, bqc_d, bkc_d, bvr_d,
          biasc_d, masku_d, ident_d, ones_d, ot_d):
    nc = tc.nc

    with ExitStack() as ctx:
        cp = ctx.enter_context(tc.tile_pool(name="const", bufs=1))
        masku_t = cp.tile([128, 128], BF16)
        ident_t = cp.tile([128, 128], BF16)
        ones_t = cp.tile([1, 128], BF16)
        biasc_t = cp.tile([128, NKT], F32)
        bqc_t = cp.tile([128, 2], F32)
        bkc_t = cp.tile([128, 2], F32)
        bvr_t = cp.tile([1, D2], BF16)
        for t_, d_ in ((masku_t, masku_d), (ident_t, ident_d),
                       (ones_t, ones_d), (biasc_t, biasc_d), (bqc_t, bqc_d),
                       (bkc_t, bkc_d), (bvr_t, bvr_d)):
            nc.sync.dma_start(t_[:], d_.ap())

        xw = ctx.enter_context(tc.tile_pool(name="xw", bufs=1))
        x8_t = xw.tile([128, 8, T], F8)
        xl_t = xw.tile([128, 8, T], F8)
        w_t = xw.tile([128, 8, 1536], F8)

        qk = ctx.enter_context(tc.tile_pool(name="qk", bufs=1))
        qt_t = qk.tile([128, 2, T], BF16)
        kt_t = qk.tile([128, 2, T], BF16)

        vtp = ctx.enter_context(tc.tile_pool(name="vt", bufs=1))
        v_t = vtp.tile([128, NKT, D2], BF16)

        pp = ctx.enter_context(tc.tile_pool(name="pp", bufs=1))
        vp = ctx.enter_context(tc.tile_pool(name="vp", bufs=1))
        st = ctx.enter_context(tc.tile_pool(name="st", bufs=6))
        osb = ctx.enter_context(tc.tile_pool(name="osb", bufs=1))

        sc_pool = ctx.enter_context(
            tc.tile_pool(name="scp", bufs=3, space="PSUM"))
        pv_pool = ctx.enter_context(
            tc.tile_pool(name="pvp", bufs=1, space="PSUM"))

        # ---- input DMA, ordered for earliest exp start ----
        nc.sync.dma_start(w_t[:, :, 0:512], wqk8_d.ap())
        nc.sync.dma_start(x8_t[:, :, 1536:2048], x8_d.ap()[:, :, 1536:2048])
        nc.sync.dma_start(w_t[:, :, 512:1024], wqkl_d.ap())
        nc.sync.dma_start(xl_t[:, :, 1536:2048], xl_d.ap()[:, :, 1536:2048])
        nc.sync.dma_start(w_t[:, :, 1024:1536], wv8l_d.ap())
        for tb in (2, 1, 0):
            nc.sync.dma_start(x8_t[:, :, tb * 512:tb * 512 + 512],
                              x8_d.ap()[:, :, tb * 512:tb * 512 + 512])
            nc.sync.dma_start(xl_t[:, :, tb * 512:tb * 512 + 512],
                              xl_d.ap()[:, :, tb * 512:tb * 512 + 512])

        # warm the exp table off the critical path
        warm_t = st.tile([1, 2], F32, name="warm")
        nc.scalar.activation(warm_t[:], ones_t[0:1, 0:2], EXP,
                             bias=0.0, scale=SCALE)

        # ---- P~ / V~ tiles (SBUF-resident until the duo's A@V) ----
        p_pair = {}   # (duo, hh, p) -> [128, 2, Wp] fp8
        p_tail = {}   # (duo, hh, kt in 14,15) -> [128, W] fp16
        vp_pair = {}  # (duo, p) -> [128, 2, 128] fp8   (both heads in free)
        vp_tail = {}  # (duo, kt) -> [128, 128] fp16
        for d in range(2):
            for hh in range(2):
                for p in range(NPAIR):
                    wp = T - 256 * p
                    p_pair[(d, hh, p)] = pp.tile(
                        [128, 2, wp], F8, tag=f"p{d}_{hh}_{p}",
                        name=f"p{d}_{hh}_{p}")
                for kt in (14, 15):
                    p_tail[(d, hh, kt)] = pp.tile(
                        [128, T - 128 * kt], F16, tag=f"pt{d}_{hh}_{kt}",
                        name=f"pt{d}_{hh}_{kt}")
            for kt in (14, 15):
                vp_tail[(d, kt)] = vp.tile(
                    [128, 128], F16, tag=f"vpt{d}_{kt}",
                    name=f"vpt{d}_{kt}")
            for p in range(NPAIR):
                vp_pair[(d, p)] = vp.tile(
                    [128, 2, 128], F8, tag=f"vp{d}_{p}",
                    name=f"vp{d}_{p}")
        # zero the odd-kt first-128 strips (masked region the exp never
        # writes); gpsimd keeps this off the busy engines
        for d in range(2):
            for hh in range(2):
                for p in range(NPAIR):
                    nc.gpsimd.memset(p_pair[(d, hh, p)][:, 1, 0:128], 0.0)

        # ---- PE helper emitters ----
        pv_tog = [0]

        def pv_tile(name):
            tag = ("pj", "ob")[pv_tog[0] % 2]
            pv_tog[0] += 1
            return pv_pool.tile([128, 512], F32, tag=tag, name=name)

        def emit_qk_chunk(duo, is_k, c):
            # one 512-col chunk of the Q^T/K^T projection for `duo`:
            # psum = X8.T@W8 + Xl.T@W8 + X8.T@Wl  (12 fp8 DoubleRow matmuls)
            w8o, wlo = (K8O, KLO) if is_k else (Q8O, QLO)
            out_t, b_t = (kt_t, bkc_t) if is_k else (qt_t, bqc_t)
            ps = pv_tile(f"qk{duo}_{int(is_k)}_{c}")
            first = True
            for xs, wo in ((x8_t, w8o), (xl_t, w8o), (x8_t, wlo)):
                for ep in range(4):
                    nc.tensor.matmul(
                        ps[:, 0:512],
                        lhsT=w_t[:, 2 * ep:2 * ep + 2,
                                 wo + duo * 128:wo + duo * 128 + 128],
                        rhs=xs[:, 2 * ep:2 * ep + 2, c * 512:c * 512 + 512],
                        start=first,
                        stop=(xs is x8_t and wo == wlo and ep == 3),
                        perf_mode=DRM,
                    )
                    first = False
            nc.vector.tensor_scalar_add(
                out_t[:, duo, c * 512:c * 512 + 512], ps[:, 0:512],
                b_t[:, duo:duo + 1])

        def emit_v_tile(kt):
            # V tile (both duos): [128 t, 256 d] = X.T@Wv*32 + 32*bv
            ps = pv_tile(f"v{kt}")
            pvs = ps[:, 0:D2]
            for xs, wo in ((x8_t, V8O), (xl_t, V8O), (x8_t, VLO)):
                for ep in range(4):
                    nc.tensor.matmul(
                        pvs,
                        lhsT=xs[:, 2 * ep:2 * ep + 2,
                                kt * 128:kt * 128 + 128],
                        rhs=w_t[:, 2 * ep:2 * ep + 2, wo:wo + D2],
                        start=(xs is x8_t and wo == V8O and ep == 0),
                        stop=False,
                        perf_mode=DRM,
                    )
            nc.tensor.matmul(pvs, lhsT=ones_t[0:1, :], rhs=bvr_t[0:1, :],
                             start=False, stop=True)
            nc.vector.tensor_copy(v_t[:, kt, :], pvs)

        def emit_scores_exp(duo, kt):
            # scores S^T[key, q] for q in [qlo, T), exp'd into P~ with
            # per-key bias -ln(c_k); accum -> rs (per-key sums r~)
            qlo = kt * 128
            w = T - qlo
            pieces = [(0, min(w, 1024))]
            if w > 1024:
                pieces.append((1024, w - 1024))
            rs_t = st.tile([128, 2], F32, tag="rs", name=f"rs{duo}_{kt}")
            sums_t = (st.tile([128, 4], F32, tag="sums", name=f"sm{duo}_{kt}")
                      if len(pieces) > 1 else None)
            for hh in range(2):
                d0 = 64 * hh
                for pi, (poff, pw) in enumerate(pieces):
                    sc = sc_pool.tile([128, 1024], F32, tag="sc", name="sc")
                    for co in range(0, pw, 512):
                        n = min(512, pw - co)
                        nc.tensor.matmul(
                            sc[:, co:co + n],
                            lhsT=kt_t[d0:d0 + 64, duo, qlo:qlo + 128],
                            rhs=qt_t[d0:d0 + 64, duo,
                                     qlo + poff + co:qlo + poff + co + n],
                            start=True,
                            stop=not (poff == 0 and co == 0),
                        )
                    if poff == 0:
                        nc.tensor.matmul(
                            sc[:, 0:128], lhsT=masku_t[:, 0:128],
                            rhs=ident_t[:], start=False, stop=True,
                            skip_group_check=True)
                    if kt >= 14:
                        dst = p_tail[(duo, hh, kt)][:, poff:poff + pw]
                    else:
                        p = kt // 2
                        par = kt % 2
                        off = 128 * par + poff
                        dst = p_pair[(duo, hh, p)][:, par, off:off + pw]
                    acc = (sums_t[:, hh * 2 + pi:hh * 2 + pi + 1]
                           if sums_t is not None else rs_t[:, hh:hh + 1])
                    nc.scalar.activation(
                        dst, sc[:, 0:pw], EXP,
                        bias=biasc_t[:, kt:kt + 1], scale=SCALE,
                        accum_out=acc)
            if sums_t is not None:
                for hh in range(2):
                    nc.vector.reduce_sum(
                        rs_t[:, hh:hh + 1], sums_t[:, hh * 2:hh * 2 + 2],
                        axis=AX)
            return rs_t

        def emit_vtilde(duo, kt, rs_t):
            # rinv = 1/r~ ; V~ = 32*v*rinv (fp8 pairs / fp16 tail)
            rinv_t = st.tile([128, 2], F32, tag="rinv", name=f"ri{duo}_{kt}")
            nc.vector.reciprocal(rinv_t[:], rs_t[:])
            for hh in range(2):
                if kt >= 14:
                    dst = vp_tail[(duo, kt)][:, 64 * hh:64 * hh + 64]
                else:
                    dst = vp_pair[(duo, kt // 2)][:, kt % 2,
                                                  64 * hh:64 * hh + 64]
                nc.vector.tensor_scalar_mul(
                    dst,
                    v_t[:, kt, duo * 128 + 64 * hh:duo * 128 + 64 * hh + 64],
                    rinv_t[:, hh:hh + 1])

        def emit_av_qtile(duo, j, ot_sb):
            # O[q, d] for q-tile j: fp8 DoubleRow, P~ stationary, V~ moving
            # with both heads side-by-side; fp16 solo for key-tiles 14, 15.
            # Out partitions = queries (always base 0).
            ob = pv_tile(f"av{duo}_{j}")
            obq = ob[:, 0:128]
            plast = min(j // 2, NPAIR - 1)
            for hh in range(2):
                for p in range(plast + 1):
                    c0 = 128 * j - 256 * p
                    nc.tensor.matmul(
                        obq[:, 64 * hh:64 * hh + 64],
                        lhsT=p_pair[(duo, hh, p)][:, :, c0:c0 + 128],
                        rhs=vp_pair[(duo, p)][:, :, 64 * hh:64 * hh + 64],
                        start=(p == 0),
                        stop=(j < 14 and p == plast),
                        perf_mode=DRM,
                        skip_group_check=True,
                    )
                for kt in (14, 15):
                    if kt > j:
                        continue
                    c0 = 128 * j - 128 * kt
                    nc.tensor.matmul(
                        obq[:, 64 * hh:64 * hh + 64],
                        lhsT=p_tail[(duo, hh, kt)][:, c0:c0 + 128],
                        rhs=vp_tail[(duo, kt)][:, 64 * hh:64 * hh + 64],
                        start=False,
                        stop=(kt == min(j, 15)),
                        skip_group_check=True,
                    )
            nc.vector.tensor_scalar_mul(
                ot_sb[:, 128 * j:128 * j + 128], obq, 1.0 / 32.0)
            nc.sync.dma_start(
                ot_d.ap()[128 * j:128 * j + 128,
                          duo * 128:duo * 128 + 128],
                ot_sb[:, 128 * j:128 * j + 128])

        # ---- schedule ----
        # duo0 fillers: own Q/K chunks, duo1 Q/K chunks, V tiles (shifted
        # +3 so kt15's V-matmul never blocks the first scores)
        qk_sched = {13: (0, False, 2), 12: (0, True, 2),
                    11: (1, False, 3), 10: (1, True, 3),
                    9: (0, False, 1), 8: (0, True, 1),
                    7: (1, False, 2), 6: (1, True, 2),
                    5: (0, False, 0), 4: (0, True, 0),
                    3: (1, False, 1), 2: (1, True, 1),
                    1: (1, False, 0), 0: (1, True, 0)}

        emit_qk_chunk(0, False, 3)
        emit_qk_chunk(0, True, 3)

        ot_sbs = [osb.tile([128, T], F32, tag=f"osb{d}", name=f"osb{d}")
                  for d in range(2)]
        # duo0's 16 A@V q-tiles, injected 2-per-kt into duo1's phase A
        av0_sched = {13: (0, 1), 12: (2, 3), 11: (4, 5), 10: (6, 7),
                     9: (8, 9), 8: (10, 11), 7: (12, 13), 6: (14, 15)}

        rs_pend = {}
        for duo in range(2):
            for kt in range(NKT - 1, -1, -1):
                rs_t = emit_scores_exp(duo, kt)
                if duo == 0:
                    rs_pend[kt] = rs_t
                    if kt in qk_sched:
                        emit_qk_chunk(*qk_sched[kt])
                    jj = kt + 3
                    if jj <= NKT - 1:
                        emit_v_tile(jj)
                        emit_vtilde(0, jj, rs_pend.pop(jj))
                    if kt == 0:
                        for jj in (2, 1, 0):
                            emit_v_tile(jj)
                            emit_vtilde(0, jj, rs_pend.pop(jj))
                else:
                    emit_vtilde(1, kt, rs_t)
                    for j in av0_sched.get(kt, ()):
                        emit_av_qtile(0, j, ot_sbs[0])
        for j in range(NKT):
            emit_av_qtile(1, j, ot_sbs[1])


def _get_module():
    if "nc" not in _CACHE:
        _CACHE["nc"] = _build_module()
    return _CACHE["nc"]


def _host_tables():
    k = np.arange(T)
    c = np.where(
        k < T - 256,
        2.0 ** np.round(0.5 * np.log2(1.031 * (T - k))),
        1.0)
    biasc = (-np.log(c)).reshape(NKT, 128).T.astype(np.float32)
    q = np.arange(128)
    masku = np.where(q[:, None] < q[None, :], NEG, 0.0).astype(np_bf16)
    ident = np.eye(128, dtype=np.float32).astype(np_bf16)
    ones = np.ones((1, 128), np.float32).astype(np_bf16)
    return biasc, masku, ident, ones


def _split8(a):
    hi = a.astype(np_f8)
    lo = (a - hi.astype(np.float32)).astype(np_f8)
    return hi, lo


def _make_in_maps(X, Wq, bq, Wk, bk, Wv, bv):
    X = np.asarray(X, np.float32)
    biasc, masku, ident, ones = _host_tables()
    in_maps = []
    for core in range(8):
        b, g = divmod(core, 4)
        rows = slice(D2 * g, D2 * g + D2)
        xt = np.ascontiguousarray(X[b].T)              # [E, T]
        x8, xl = _split8(xt)
        x8 = np.ascontiguousarray(x8.reshape(8, 128, T).transpose(1, 0, 2))
        xl = np.ascontiguousarray(xl.reshape(8, 128, T).transpose(1, 0, 2))

        def wprep(Wfull):
            ws = np.asarray(Wfull)[rows].T.astype(np.float32) * WS  # [E, 256]
            return _split8(ws)

        wq8, wql = wprep(Wq)
        wk8, wkl = wprep(Wk)
        wv8, wvl = wprep(Wv)
        wqk8 = np.concatenate([wq8, wk8], axis=1)      # [E, 512]
        wqkl = np.concatenate([wql, wkl], axis=1)
        wv8l = np.concatenate([wv8, wvl], axis=1)

        def dr3(w):  # [E, 512] -> [128, 8, 512]
            return np.ascontiguousarray(
                w.reshape(8, 128, 512).transpose(1, 0, 2))

        in_maps.append({
            "x8": x8, "xl": xl,
            "wqk8": dr3(wqk8), "wqkl": dr3(wqkl), "wv8l": dr3(wv8l),
            "bqc": np.ascontiguousarray(
                (WS * np.asarray(bq)[rows]).reshape(2, 128).T
            ).astype(np.float32),
            "bkc": np.ascontiguousarray(
                (WS * np.asarray(bk)[rows]).reshape(2, 128).T
            ).astype(np.float32),
            "bvr": (WS * np.asarray(bv)[rows]).reshape(1, D2).astype(np_bf16),
            "biasc": biasc, "masku": masku, "ident": ident, "ones": ones,
        })
    return in_maps


def kernel(X, Wq, bq, Wk, bk, Wv, bv, **kw):
    in_maps = _make_in_maps(X, Wq, bq, Wk, bk, Wv, bv)
    nc = _get_module()
    res = run_bass_kernel_spmd(nc, in_maps, core_ids=list(range(8)), **kw)
    _CACHE["last_res"] = res
    out = np.zeros((B, T, E), np.float32)
    for c in range(8):
        b, g = divmod(c, 4)
        out[b, :, D2 * g:D2 * g + D2] = res.results[c]["ot"]
    return out


if __name__ == "__main__":
    _get_module()
    print("module built ok")


# revision 14
# speedup vs baseline: 1.4813x; 1.1203x over previous
"""Multi-head attention (softmax over the QUERY axis) for Trainium2, 8 cores.

Reference (B=2, T=2048, E=1024, H=16, HD=64):
    q = X@Wq.T+bq ; k = ... ; v = ...   (per-head split)
    s = (q k^T)/sqrt(E), causal mask (key > query -> -inf)
    attn = softmax(s, axis=QUERY)  -> normalizes each key COLUMN over queries
    out  = attn @ v

Sharding: core c = (batch c//4, head-group c%4 of 4 heads = 2 duos).  No
collectives.

Math per core (d2=256 output dims as O^T [256, T]):
  Projections run in fp8e4 DoubleRow with hi+lo error compensation:
  W' = 32*W split W8+Wl (fp8), X split X8+Xl; q32 ~= X8@W8 + Xl@W8 + X8@Wl
  (+32b).  Scores use bf16 Q/K: s_psum = 1024*s; exp applies
  scale=1/32768 and per-key bias -ln(c_k), c_k ~ sqrt(E[r_k]) (any c_k is
  mathematically exact: V-scaling self-corrects).  P~=P/c_k stored fp8
  (keys < 1792) / fp16 (last 256 keys); V~ = 32*v*c/r.  A@V runs fp8
  DoubleRow over PAIRS of key-tiles (contraction 256); final PSUM->SBUF
  copy multiplies by 1/32.

Schedule: per duo, key-tiles processed DESCENDING (small score tiles
first -> exp starts ~7us in).  All P~/V~ live in SBUF; O^T accumulation
(phase B) runs bank-at-a-time overlapped with the next duo's phase A.
PSUM: 3x[128,1024] score slots + 2x[128,512] proj/V/O slots.
"""

import math
from contextlib import ExitStack

import numpy as np
import ml_dtypes

import concourse.bacc as bacc
import concourse.mybir as mybir
import concourse.tile as tile
from concourse.bass_utils import run_bass_kernel_spmd

B, T, E, H = 2, 2048, 1024, 16
D2 = 256              # output dims per core (4 heads)
NKT = 16              # key tiles of 128
NPAIR = 7             # fp8 DoubleRow pairs: kts (2p, 2p+1), p=0..6; kts 14,15 fp16
F32 = mybir.dt.float32
BF16 = mybir.dt.bfloat16
F16 = mybir.dt.float16
F8 = mybir.dt.float8e4
DRM = mybir.MatmulPerfMode.DoubleRow
EXP = mybir.ActivationFunctionType.Exp
AX = mybir.AxisListType.X
SCALE = 1.0 / 32768.0   # 1/sqrt(E) / 32 / 32  (both Q,K carry 32x weights)
NEG = -1.0e30
WS = 32.0               # weight prescale
np_f8 = ml_dtypes.float8_e4m3
np_bf16 = ml_dtypes.bfloat16

# w_t column-block offsets (per e-chunk, 1536 wide)
Q8O, K8O, QLO, KLO, V8O, VLO = 0, 256, 512, 768, 1024, 1280

_CACHE = {}


def _build_module():
    nc = bacc.Bacc("TRN2", target_bir_lowering=False, debug=False)

    x8_d = nc.dram_tensor("x8", [128, 8, T], F8, kind="ExternalInput")
    xl_d = nc.dram_tensor("xl", [128, 8, T], F8, kind="ExternalInput")
    wqk8_d = nc.dram_tensor("wqk8", [128, 8, 512], F8, kind="ExternalInput")
    wqkl_d = nc.dram_tensor("wqkl", [128, 8, 512], F8, kind="ExternalInput")
    wv8l_d = nc.dram_tensor("wv8l", [128, 8, 512], F8, kind="ExternalInput")
    # merged consts: fewer DMA instructions (each costs ~625ns of HWDGE)
    cf_d = nc.dram_tensor("cf", [128, NKT + 4], F32, kind="ExternalInput")
    mi_d = nc.dram_tensor("mi", [128, 256], BF16, kind="ExternalInput")
    ob_d = nc.dram_tensor("ob", [1, 128 + D2], BF16, kind="ExternalInput")
    ot_d = nc.dram_tensor("ot", [T, D2], F32, kind="ExternalOutput")

    with tile.TileContext(nc) as tc:
        _body(tc, x8_d, xl_d, wqk8_d, wqkl_d, wv8l_d, cf_d, mi_d, ob_d, ot_d)
    nc.compile()
    return nc


def _body(tc, x8_d, xl_d, wqk8_d, wqkl_d, wv8l_d, cf_d, mi_d, ob_d, ot_d):
    nc = tc.nc

    with ExitStack() as ctx:
        cp = ctx.enter_context(tc.tile_pool(name="const", bufs=1))
        cf_t = cp.tile([128, NKT + 4], F32)     # [biasc | bqc | bkc]
        mi_t = cp.tile([128, 256], BF16)        # [masku | ident]
        ob_t = cp.tile([1, 128 + D2], BF16)     # [ones | bvr]
        biasc_t = cf_t[:, 0:NKT]
        bqc_t = cf_t[:, NKT:NKT + 2]
        bkc_t = cf_t[:, NKT + 2:NKT + 4]
        masku_t = mi_t[:, 0:128]
        ident_t = mi_t[:, 128:256]
        ones_t = ob_t[:, 0:128]
        bvr_t = ob_t[:, 128:128 + D2]
        nc.sync.dma_start(ob_t[:], ob_d.ap())

        xw = ctx.enter_context(tc.tile_pool(name="xw", bufs=1))
        x8_t = xw.tile([128, 8, T], F8)
        xl_t = xw.tile([128, 8, T], F8)
        w_t = xw.tile([128, 8, 1536], F8)

        qk = ctx.enter_context(tc.tile_pool(name="qk", bufs=1))
        qt_t = qk.tile([128, 2, T], BF16)
        kt_t = qk.tile([128, 2, T], BF16)

        vtp = ctx.enter_context(tc.tile_pool(name="vt", bufs=1))
        v_t = vtp.tile([128, NKT, D2], BF16)

        pp = ctx.enter_context(tc.tile_pool(name="pp", bufs=1))
        vp = ctx.enter_context(tc.tile_pool(name="vp", bufs=1))
        st = ctx.enter_context(tc.tile_pool(name="st", bufs=6))
        osb = ctx.enter_context(tc.tile_pool(name="osb", bufs=1))

        sc_pool = ctx.enter_context(
            tc.tile_pool(name="scp", bufs=3, space="PSUM"))
        pv_pool = ctx.enter_context(
            tc.tile_pool(name="pvp", bufs=1, space="PSUM"))

        # ---- input DMA, ordered for earliest exp start ----
        nc.sync.dma_start(w_t[:, :, 0:512], wqk8_d.ap())
        nc.sync.dma_start(x8_t[:, :, 1536:2048], x8_d.ap()[:, :, 1536:2048])
        nc.sync.dma_start(w_t[:, :, 512:1024], wqkl_d.ap())
        nc.sync.dma_start(xl_t[:, :, 1536:2048], xl_d.ap()[:, :, 1536:2048])
        nc.sync.dma_start(mi_t[:], mi_d.ap())
        nc.sync.dma_start(cf_t[:], cf_d.ap())
        nc.sync.dma_start(w_t[:, :, 1024:1536], wv8l_d.ap())
        for tb in (2, 1, 0):
            nc.sync.dma_start(x8_t[:, :, tb * 512:tb * 512 + 512],
                              x8_d.ap()[:, :, tb * 512:tb * 512 + 512])
            nc.sync.dma_start(xl_t[:, :, tb * 512:tb * 512 + 512],
                              xl_d.ap()[:, :, tb * 512:tb * 512 + 512])

        # warm the exp table off the critical path
        warm_t = st.tile([1, 2], F32, name="warm")
        nc.scalar.activation(warm_t[:], ones_t[0:1, 0:2], EXP,
                             bias=0.0, scale=SCALE)

        # ---- P~ / V~ tiles (SBUF-resident until the duo's A@V) ----
        p_pair = {}   # (duo, hh, p) -> [128, 2, Wp] fp8
        p_tail = {}   # (duo, hh, kt in 14,15) -> [128, W] fp16
        vp_pair = {}  # (duo, p) -> [128, 2, 128] fp8   (both heads in free)
        vp_tail = {}  # (duo, kt) -> [128, 128] fp16
        for d in range(2):
            for hh in range(2):
                for p in range(NPAIR):
                    wp = T - 256 * p
                    p_pair[(d, hh, p)] = pp.tile(
                        [128, 2, wp], F8, tag=f"p{d}_{hh}_{p}",
                        name=f"p{d}_{hh}_{p}")
                for kt in (14, 15):
                    p_tail[(d, hh, kt)] = pp.tile(
                        [128, T - 128 * kt], F16, tag=f"pt{d}_{hh}_{kt}",
                        name=f"pt{d}_{hh}_{kt}")
            for kt in (14, 15):
                vp_tail[(d, kt)] = vp.tile(
                    [128, 128], F16, tag=f"vpt{d}_{kt}",
                    name=f"vpt{d}_{kt}")
            for p in range(NPAIR):
                vp_pair[(d, p)] = vp.tile(
                    [128, 2, 128], F8, tag=f"vp{d}_{p}",
                    name=f"vp{d}_{p}")
        # zero the odd-kt first-128 strips (masked region the exp never
        # writes); gpsimd keeps this off the busy engines
        for d in range(2):
            for hh in range(2):
                for p in range(NPAIR):
                    nc.gpsimd.memset(p_pair[(d, hh, p)][:, 1, 0:128], 0.0)

        # ---- PE helper emitters ----
        pv_tog = [0]

        def pv_tile(name):
            tag = ("pj", "ob")[pv_tog[0] % 2]
            pv_tog[0] += 1
            return pv_pool.tile([128, 512], F32, tag=tag, name=name)

        def emit_qk_chunk(duo, is_k, c):
            # one 512-col chunk of the Q^T/K^T projection for `duo`:
            # psum = X8.T@W8 + Xl.T@W8 + X8.T@Wl  (12 fp8 DoubleRow matmuls)
            w8o, wlo = (K8O, KLO) if is_k else (Q8O, QLO)
            out_t, b_t = (kt_t, bkc_t) if is_k else (qt_t, bqc_t)
            ps = pv_tile(f"qk{duo}_{int(is_k)}_{c}")
            first = True
            for xs, wo in ((x8_t, w8o), (xl_t, w8o), (x8_t, wlo)):
                for ep in range(4):
                    nc.tensor.matmul(
                        ps[:, 0:512],
                        lhsT=w_t[:, 2 * ep:2 * ep + 2,
                                 wo + duo * 128:wo + duo * 128 + 128],
                        rhs=xs[:, 2 * ep:2 * ep + 2, c * 512:c * 512 + 512],
                        start=first,
                        stop=(xs is x8_t and wo == wlo and ep == 3),
                        perf_mode=DRM,
                    )
                    first = False
            nc.vector.tensor_scalar_add(
                out_t[:, duo, c * 512:c * 512 + 512], ps[:, 0:512],
                b_t[:, duo:duo + 1])

        def emit_v_tile(kt):
            # V tile (both duos): [128 t, 256 d] = X.T@Wv*32 + 32*bv
            ps = pv_tile(f"v{kt}")
            pvs = ps[:, 0:D2]
            for xs, wo in ((x8_t, V8O), (xl_t, V8O), (x8_t, VLO)):
                for ep in range(4):
                    nc.tensor.matmul(
                        pvs,
                        lhsT=xs[:, 2 * ep:2 * ep + 2,
                                kt * 128:kt * 128 + 128],
                        rhs=w_t[:, 2 * ep:2 * ep + 2, wo:wo + D2],
                        start=(xs is x8_t and wo == V8O and ep == 0),
                        stop=False,
                        perf_mode=DRM,
                    )
            nc.tensor.matmul(pvs, lhsT=ones_t[0:1, :], rhs=bvr_t[0:1, :],
                             start=False, stop=True)
            nc.vector.tensor_copy(v_t[:, kt, :], pvs)

        def emit_scores_exp(duo, kt):
            # scores S^T[key, q] for q in [qlo, T), exp'd into P~ with
            # per-key bias -ln(c_k); accum -> rs (per-key sums r~)
            qlo = kt * 128
            w = T - qlo
            pieces = [(0, min(w, 1024))]
            if w > 1024:
                pieces.append((1024, w - 1024))
            rs_t = st.tile([128, 2], F32, tag="rs", name=f"rs{duo}_{kt}")
            sums_t = (st.tile([128, 4], F32, tag="sums", name=f"sm{duo}_{kt}")
                      if len(pieces) > 1 else None)
            for hh in range(2):
                d0 = 64 * hh
                for pi, (poff, pw) in enumerate(pieces):
                    sc = sc_pool.tile([128, 1024], F32, tag="sc", name="sc")
                    for co in range(0, pw, 512):
                        n = min(512, pw - co)
                        nc.tensor.matmul(
                            sc[:, co:co + n],
                            lhsT=kt_t[d0:d0 + 64, duo, qlo:qlo + 128],
                            rhs=qt_t[d0:d0 + 64, duo,
                                     qlo + poff + co:qlo + poff + co + n],
                            start=True,
                            stop=not (poff == 0 and co == 0),
                        )
                    if poff == 0:
                        nc.tensor.matmul(
                            sc[:, 0:128], lhsT=masku_t[:, 0:128],
                            rhs=ident_t[:], start=False, stop=True,
                            skip_group_check=True)
                    if kt >= 14:
                        dst = p_tail[(duo, hh, kt)][:, poff:poff + pw]
                    else:
                        p = kt // 2
                        par = kt % 2
                        off = 128 * par + poff
                        dst = p_pair[(duo, hh, p)][:, par, off:off + pw]
                    acc = (sums_t[:, hh * 2 + pi:hh * 2 + pi + 1]
                           if sums_t is not None else rs_t[:, hh:hh + 1])
                    nc.scalar.activation(
                        dst, sc[:, 0:pw], EXP,
                        bias=biasc_t[:, kt:kt + 1], scale=SCALE,
                        accum_out=acc)
            if sums_t is not None:
                for hh in range(2):
                    nc.vector.reduce_sum(
                        rs_t[:, hh:hh + 1], sums_t[:, hh * 2:hh * 2 + 2],
                        axis=AX)
            return rs_t

        def emit_vtilde(duo, kt, rs_t):
            # rinv = 1/r~ ; V~ = 32*v*rinv (fp8 pairs / fp16 tail)
            rinv_t = st.tile([128, 2], F32, tag="rinv", name=f"ri{duo}_{kt}")
            nc.vector.reciprocal(rinv_t[:], rs_t[:])
            for hh in range(2):
                if kt >= 14:
                    dst = vp_tail[(duo, kt)][:, 64 * hh:64 * hh + 64]
                else:
                    dst = vp_pair[(duo, kt // 2)][:, kt % 2,
                                                  64 * hh:64 * hh + 64]
                nc.vector.tensor_scalar_mul(
                    dst,
                    v_t[:, kt, duo * 128 + 64 * hh:duo * 128 + 64 * hh + 64],
                    rinv_t[:, hh:hh + 1])

        def emit_av_qtile(duo, j, ot_sb):
            # O[q, d] for q-tile j: fp8 DoubleRow, P~ stationary, V~ moving
            # with both heads side-by-side; fp16 solo for key-tiles 14, 15.
            # Out partitions = queries (always base 0).
            ob = pv_tile(f"av{duo}_{j}")
            obq = ob[:, 0:128]
            plast = min(j // 2, NPAIR - 1)
            for hh in range(2):
                for p in range(plast + 1):
                    c0 = 128 * j - 256 * p
                    nc.tensor.matmul(
                        obq[:, 64 * hh:64 * hh + 64],
                        lhsT=p_pair[(duo, hh, p)][:, :, c0:c0 + 128],
                        rhs=vp_pair[(duo, p)][:, :, 64 * hh:64 * hh + 64],
                        start=(p == 0),
                        stop=(j < 14 and p == plast),
                        perf_mode=DRM,
                        skip_group_check=True,
                    )
                for kt in (14, 15):
                    if kt > j:
                        continue
                    c0 = 128 * j - 128 * kt
                    nc.tensor.matmul(
                        obq[:, 64 * hh:64 * hh + 64],
                        lhsT=p_tail[(duo, hh, kt)][:, c0:c0 + 128],
                        rhs=vp_tail[(duo, kt)][:, 64 * hh:64 * hh + 64],
                        start=False,
                        stop=(kt == min(j, 15)),
                        skip_group_check=True,
                    )
            nc.vector.tensor_scalar_mul(
                ot_sb[:, 128 * j:128 * j + 128], obq, 1.0 / 32.0)
            nc.sync.dma_start(
                ot_d.ap()[128 * j:128 * j + 128,
                          duo * 128:duo * 128 + 128],
                ot_sb[:, 128 * j:128 * j + 128])

        # ---- schedule ----
        # duo0 fillers: own Q/K chunks, duo1 Q/K chunks, V tiles (shifted
        # +3 so kt15's V-matmul never blocks the first scores)
        qk_sched = {13: (0, False, 2), 12: (0, True, 2),
                    11: (1, False, 3), 10: (1, True, 3),
                    9: (0, False, 1), 8: (0, True, 1),
                    7: (1, False, 2), 6: (1, True, 2),
                    5: (0, False, 0), 4: (0, True, 0),
                    3: (1, False, 1), 2: (1, True, 1),
                    1: (1, False, 0), 0: (1, True, 0)}

        emit_qk_chunk(0, False, 3)
        emit_qk_chunk(0, True, 3)

        ot_sbs = [osb.tile([128, T], F32, tag=f"osb{d}", name=f"osb{d}")
                  for d in range(2)]
        # duo0's 16 A@V q-tiles, injected early into duo1's phase A
        av0_sched = {0: (0, 1, 2), 1: (3, 4, 5), 2: (6, 7, 8),
                     3: (9, 10, 11), 4: (12, 13), 5: (14, 15)}

        # duo0: key-tiles DESCENDING (small score tiles first -> early exp
        # start under partial DMA); duo1: ASCENDING so its own A@V q-tiles
        # interleave as soon as their key-pairs complete (short tail).
        rs_pend = {}
        for kt in range(NKT - 1, -1, -1):
            rs_t = emit_scores_exp(0, kt)
            rs_pend[kt] = rs_t
            if kt in qk_sched:
                emit_qk_chunk(*qk_sched[kt])
            jj = kt + 3
            if jj <= NKT - 1:
                emit_v_tile(jj)
                emit_vtilde(0, jj, rs_pend.pop(jj))
            if kt == 0:
                for jj in (2, 1, 0):
                    emit_v_tile(jj)
                    emit_vtilde(0, jj, rs_pend.pop(jj))
        for kt in range(NKT):
            rs_t = emit_scores_exp(1, kt)
            emit_vtilde(1, kt, rs_t)
            for j in av0_sched.get(kt, ()):
                emit_av_qtile(0, j, ot_sbs[0])
            if kt % 2 == 1 and kt <= 13:
                emit_av_qtile(1, kt - 1, ot_sbs[1])
                emit_av_qtile(1, kt, ot_sbs[1])
            elif kt >= 14:
                emit_av_qtile(1, kt, ot_sbs[1])


def _get_module():
    if "nc" not in _CACHE:
        _CACHE["nc"] = _build_module()
    return _CACHE["nc"]


def _host_tables():
    k = np.arange(T)
    c = np.where(
        k < T - 256,
        2.0 ** np.round(0.5 * np.log2(1.031 * (T - k))),
        1.0)
    biasc = (-np.log(c)).reshape(NKT, 128).T.astype(np.float32)
    qi = np.arange(128)
    masku = np.where(qi[:, None] < qi[None, :], NEG, 0.0).astype(np_bf16)
    ident = np.eye(128, dtype=np.float32).astype(np_bf16)
    ones = np.ones((1, 128), np.float32).astype(np_bf16)
    return biasc, masku, ident, ones


def _split8(a):
    hi = a.astype(np_f8)
    lo = (a - hi.astype(np.float32)).astype(np_f8)
    return hi, lo


def _make_in_maps(X, Wq, bq, Wk, bk, Wv, bv):
    X = np.asarray(X, np.float32)
    biasc, masku, ident, ones = _host_tables()
    in_maps = []
    for core in range(8):
        b, g = divmod(core, 4)
        rows = slice(D2 * g, D2 * g + D2)
        xt = np.ascontiguousarray(X[b].T)              # [E, T]
        x8, xl = _split8(xt)
        x8 = np.ascontiguousarray(x8.reshape(8, 128, T).transpose(1, 0, 2))
        xl = np.ascontiguousarray(xl.reshape(8, 128, T).transpose(1, 0, 2))

        def wprep(Wfull):
            ws = np.asarray(Wfull)[rows].T.astype(np.float32) * WS  # [E, 256]
            return _split8(ws)

        wq8, wql = wprep(Wq)
        wk8, wkl = wprep(Wk)
        wv8, wvl = wprep(Wv)
        wqk8 = np.concatenate([wq8, wk8], axis=1)      # [E, 512]
        wqkl = np.concatenate([wql, wkl], axis=1)
        wv8l = np.concatenate([wv8, wvl], axis=1)

        def dr3(w):  # [E, 512] -> [128, 8, 512]
            return np.ascontiguousarray(
                w.reshape(8, 128, 512).transpose(1, 0, 2))

        bqc = (WS * np.asarray(bq)[rows]).reshape(2, 128).T
        bkc = (WS * np.asarray(bk)[rows]).reshape(2, 128).T
        cf = np.concatenate([biasc, bqc, bkc], axis=1).astype(np.float32)
        mi = np.concatenate([masku, ident], axis=1)
        ob = np.concatenate(
            [ones, (WS * np.asarray(bv)[rows]).reshape(1, D2).astype(np_bf16)],
            axis=1)
        in_maps.append({
            "x8": x8, "xl": xl,
            "wqk8": dr3(wqk8), "wqkl": dr3(wqkl), "wv8l": dr3(wv8l),
            "cf": np.ascontiguousarray(cf),
            "mi": np.ascontiguousarray(mi),
            "ob": np.ascontiguousarray(ob),
        })
    return in_maps


def kernel(X, Wq, bq, Wk, bk, Wv, bv, **kw):
    in_maps = _make_in_maps(X, Wq, bq, Wk, bk, Wv, bv)
    nc = _get_module()
    res = run_bass_kernel_spmd(nc, in_maps, core_ids=list(range(8)), **kw)
    _CACHE["last_res"] = res
    out = np.zeros((B, T, E), np.float32)
    for c in range(8):
        b, g = divmod(c, 4)
        out[b, :, D2 * g:D2 * g + D2] = res.results[c]["ot"]
    return out


if __name__ == "__main__":
    _get_module()
    print("module built ok")


# revision 20
# speedup vs baseline: 1.5251x; 1.0295x over previous
"""Multi-head attention (softmax over the QUERY axis) for Trainium2, 8 cores.

Reference (B=2, T=2048, E=1024, H=16, HD=64):
    q = X@Wq.T+bq ; k = ... ; v = ...   (per-head split)
    s = (q k^T)/sqrt(E), causal mask (key > query -> -inf)
    attn = softmax(s, axis=QUERY)  -> normalizes each key COLUMN over queries
    out  = attn @ v

Sharding: core c = (batch c//4, head-group c%4 of 4 heads = 2 duos).  No
collectives.

Math per core (d2=256 output dims as O^T [256, T]):
  Projections run in fp8e4 DoubleRow with hi+lo error compensation:
  W' = 32*W split W8+Wl (fp8), X split X8+Xl; q32 ~= X8@W8 + Xl@W8 + X8@Wl
  (+32b).  Scores use bf16 Q/K: s_psum = 1024*s; exp applies
  scale=1/32768 and per-key bias -ln(c_k), c_k ~ sqrt(E[r_k]) (any c_k is
  mathematically exact: V-scaling self-corrects).  P~=P/c_k stored fp8
  (keys < 1792) / fp16 (last 256 keys); V~ = 32*v*c/r.  A@V runs fp8
  DoubleRow over PAIRS of key-tiles (contraction 256); final PSUM->SBUF
  copy multiplies by 1/32.

Schedule: per duo, key-tiles processed DESCENDING (small score tiles
first -> exp starts ~7us in).  All P~/V~ live in SBUF; O^T accumulation
(phase B) runs bank-at-a-time overlapped with the next duo's phase A.
PSUM: 3x[128,1024] score slots + 2x[128,512] proj/V/O slots.
"""

import math
from contextlib import ExitStack

import numpy as np
import ml_dtypes

import concourse.bacc as bacc
import concourse.mybir as mybir
import concourse.tile as tile
from concourse.bass_utils import run_bass_kernel_spmd

B, T, E, H = 2, 2048, 1024, 16
D2 = 256              # output dims per core (4 heads)
NKT = 16              # key tiles of 128
NPAIR = 7             # fp8 DoubleRow pairs: kts (2p, 2p+1), p=0..6; kts 14,15 fp16
F32 = mybir.dt.float32
BF16 = mybir.dt.bfloat16
F16 = mybir.dt.float16
F8 = mybir.dt.float8e4
DRM = mybir.MatmulPerfMode.DoubleRow
EXP = mybir.ActivationFunctionType.Exp
AX = mybir.AxisListType.X
SCALE = 1.0 / 32768.0   # 1/sqrt(E) / 32 / 32  (both Q,K carry 32x weights)
NEG = -1.0e30
WS = 32.0               # weight prescale
np_f8 = ml_dtypes.float8_e4m3
np_bf16 = ml_dtypes.bfloat16

# w_t column-block offsets (per e-chunk, 1536 wide)
Q8O, K8O, QLO, KLO, V8O, VLO = 0, 256, 512, 768, 1024, 1280

_CACHE = {}


def _build_module():
    nc = bacc.Bacc("TRN2", target_bir_lowering=False, debug=False)

    x8_d = nc.dram_tensor("x8", [128, 8, T], F8, kind="ExternalInput")
    xl_d = nc.dram_tensor("xl", [128, 8, T], F8, kind="ExternalInput")
    wqk8_d = nc.dram_tensor("wqk8", [128, 8, 512], F8, kind="ExternalInput")
    wqkl_d = nc.dram_tensor("wqkl", [128, 8, 512], F8, kind="ExternalInput")
    wv8l_d = nc.dram_tensor("wv8l", [128, 8, 512], F8, kind="ExternalInput")
    # merged consts: fewer DMA instructions (each costs ~625ns of HWDGE)
    cf_d = nc.dram_tensor("cf", [128, NKT + 4], F32, kind="ExternalInput")
    mi_d = nc.dram_tensor("mi", [128, 256], BF16, kind="ExternalInput")
    ob_d = nc.dram_tensor("ob", [1, 128 + D2], BF16, kind="ExternalInput")
    ot_d = nc.dram_tensor("ot", [T, D2], F32, kind="ExternalOutput")

    with tile.TileContext(nc) as tc:
        _body(tc, x8_d, xl_d, wqk8_d, wqkl_d, wv8l_d, cf_d, mi_d, ob_d, ot_d)
    nc.compile()
    return nc


def _body(tc, x8_d, xl_d, wqk8_d, wqkl_d, wv8l_d, cf_d, mi_d, ob_d, ot_d):
    nc = tc.nc

    with ExitStack() as ctx:
        cp = ctx.enter_context(tc.tile_pool(name="const", bufs=1))
        cf_t = cp.tile([128, NKT + 4], F32)     # [biasc | bqc | bkc]
        mi_t = cp.tile([128, 256], BF16)        # [masku | ident]
        ob_t = cp.tile([1, 128 + D2], BF16)     # [ones | bvr]
        biasc_t = cf_t[:, 0:NKT]
        bqc_t = cf_t[:, NKT:NKT + 2]
        bkc_t = cf_t[:, NKT + 2:NKT + 4]
        masku_t = mi_t[:, 0:128]
        ident_t = mi_t[:, 128:256]
        ones_t = ob_t[:, 0:128]
        bvr_t = ob_t[:, 128:128 + D2]
        nc.sync.dma_start(ob_t[:], ob_d.ap())

        xw = ctx.enter_context(tc.tile_pool(name="xw", bufs=1))
        x8_t = xw.tile([128, 8, T], F8)
        xl_t = xw.tile([128, 8, T], F8)
        w_t = xw.tile([128, 8, 1536], F8)

        qk = ctx.enter_context(tc.tile_pool(name="qk", bufs=1))
        qt_t = qk.tile([128, 2, T], BF16)
        kt_t = qk.tile([128, 2, T], BF16)

        vtp = ctx.enter_context(tc.tile_pool(name="vt", bufs=1))
        v_t = vtp.tile([128, NKT, D2], BF16)

        pp = ctx.enter_context(tc.tile_pool(name="pp", bufs=1))
        vp = ctx.enter_context(tc.tile_pool(name="vp", bufs=1))
        st = ctx.enter_context(tc.tile_pool(name="st", bufs=6))
        osb = ctx.enter_context(tc.tile_pool(name="osb", bufs=1))

        sc_pool = ctx.enter_context(
            tc.tile_pool(name="scp", bufs=2, space="PSUM"))
        pv_pool = ctx.enter_context(
            tc.tile_pool(name="pvp", bufs=1, space="PSUM"))

        # ---- input DMA, ordered for earliest exp start ----
        nc.sync.dma_start(w_t[:, :, 0:512], wqk8_d.ap())
        nc.sync.dma_start(x8_t[:, :, 1536:2048], x8_d.ap()[:, :, 1536:2048])
        nc.sync.dma_start(w_t[:, :, 512:1024], wqkl_d.ap())
        nc.sync.dma_start(xl_t[:, :, 1536:2048], xl_d.ap()[:, :, 1536:2048])
        nc.sync.dma_start(mi_t[:], mi_d.ap())
        nc.sync.dma_start(cf_t[:], cf_d.ap())
        nc.sync.dma_start(w_t[:, :, 1024:1536], wv8l_d.ap())
        for tb in (2, 1, 0):
            nc.sync.dma_start(x8_t[:, :, tb * 512:tb * 512 + 512],
                              x8_d.ap()[:, :, tb * 512:tb * 512 + 512])
            nc.sync.dma_start(xl_t[:, :, tb * 512:tb * 512 + 512],
                              xl_d.ap()[:, :, tb * 512:tb * 512 + 512])

        # warm the exp table off the critical path
        warm_t = st.tile([1, 2], F32, name="warm")
        nc.scalar.activation(warm_t[:], ones_t[0:1, 0:2], EXP,
                             bias=0.0, scale=SCALE)

        # ---- P~ / V~ tiles (SBUF-resident until the duo's A@V) ----
        p_pair = {}   # (duo, hh, p) -> [128, 2, Wp] fp8
        p_tail = {}   # (duo, hh, kt in 14,15) -> [128, W] fp16
        vp_pair = {}  # (duo, p) -> [128, 2, 128] fp8   (both heads in free)
        vp_tail = {}  # (duo, kt) -> [128, 128] fp16
        for d in range(2):
            for hh in range(2):
                for p in range(NPAIR):
                    wp = T - 256 * p
                    p_pair[(d, hh, p)] = pp.tile(
                        [128, 2, wp], F8, tag=f"p{d}_{hh}_{p}",
                        name=f"p{d}_{hh}_{p}")
                for kt in (14, 15):
                    p_tail[(d, hh, kt)] = pp.tile(
                        [128, T - 128 * kt], F16, tag=f"pt{d}_{hh}_{kt}",
                        name=f"pt{d}_{hh}_{kt}")
            for kt in (14, 15):
                vp_tail[(d, kt)] = vp.tile(
                    [128, 128], F16, tag=f"vpt{d}_{kt}",
                    name=f"vpt{d}_{kt}")
            for p in range(NPAIR):
                vp_pair[(d, p)] = vp.tile(
                    [128, 2, 128], F8, tag=f"vp{d}_{p}",
                    name=f"vp{d}_{p}")
        # zero the odd-kt first-128 strips (masked region the exp never
        # writes); gpsimd keeps this off the busy engines
        for d in range(2):
            for hh in range(2):
                for p in range(NPAIR):
                    nc.gpsimd.memset(p_pair[(d, hh, p)][:, 1, 0:128], 0.0)

        # ---- PE helper emitters ----
        pv_tog = [0]

        def pv_tile(name):
            tag = ("pj", "ob")[pv_tog[0] % 2]
            pv_tog[0] += 1
            return pv_pool.tile([128, 512], F32, tag=tag, name=name)

        def emit_qk_chunk(duo, is_k, c, off=0, n=512):
            # one chunk of the Q^T/K^T projection for `duo`:
            # psum = X8.T@W8 + Xl.T@W8 + X8.T@Wl  (12 fp8 DoubleRow matmuls)
            w8o, wlo = (K8O, KLO) if is_k else (Q8O, QLO)
            out_t, b_t = (kt_t, bkc_t) if is_k else (qt_t, bqc_t)
            c0 = c * 512 + off
            ps = pv_tile(f"qk{duo}_{int(is_k)}_{c}_{off}")
            first = True
            for xs, wo in ((x8_t, w8o), (xl_t, w8o), (x8_t, wlo)):
                for ep in range(4):
                    nc.tensor.matmul(
                        ps[:, 0:n],
                        lhsT=w_t[:, 2 * ep:2 * ep + 2,
                                 wo + duo * 128:wo + duo * 128 + 128],
                        rhs=xs[:, 2 * ep:2 * ep + 2, c0:c0 + n],
                        start=first,
                        stop=(xs is x8_t and wo == wlo and ep == 3),
                        perf_mode=DRM,
                    )
                    first = False
            nc.vector.tensor_scalar_add(
                out_t[:, duo, c0:c0 + n], ps[:, 0:n],
                b_t[:, duo:duo + 1])

        def emit_v_tile(kt):
            # V tile (both duos): [128 t, 256 d] = X.T@Wv*32 + 32*bv
            ps = pv_tile(f"v{kt}")
            pvs = ps[:, 0:D2]
            for xs, wo in ((x8_t, V8O), (xl_t, V8O), (x8_t, VLO)):
                for ep in range(4):
                    nc.tensor.matmul(
                        pvs,
                        lhsT=xs[:, 2 * ep:2 * ep + 2,
                                kt * 128:kt * 128 + 128],
                        rhs=w_t[:, 2 * ep:2 * ep + 2, wo:wo + D2],
                        start=(xs is x8_t and wo == V8O and ep == 0),
                        stop=False,
                        perf_mode=DRM,
                    )
            nc.tensor.matmul(pvs, lhsT=ones_t[0:1, :], rhs=bvr_t[0:1, :],
                             start=False, stop=True)
            nc.vector.tensor_copy(v_t[:, kt, :], pvs)

        def emit_scores_exp(duo, kt):
            # scores S^T[key, q] for q in [qlo, T), exp'd into P~ with
            # per-key bias -ln(c_k); accum -> rs (per-key sums r~)
            qlo = kt * 128
            w = T - qlo
            pieces = [(0, min(w, 1536))]
            if w > 1536:
                pieces.append((1536, w - 1536))
            rs_t = st.tile([128, 2], F32, tag="rs", name=f"rs{duo}_{kt}")
            sums_t = (st.tile([128, 4], F32, tag="sums", name=f"sm{duo}_{kt}")
                      if len(pieces) > 1 else None)
            for hh in range(2):
                d0 = 64 * hh
                for pi, (poff, pw) in enumerate(pieces):
                    sc = sc_pool.tile([128, 1536], F32, tag="sc", name="sc")
                    for co in range(0, pw, 512):
                        n = min(512, pw - co)
                        nc.tensor.matmul(
                            sc[:, co:co + n],
                            lhsT=kt_t[d0:d0 + 64, duo, qlo:qlo + 128],
                            rhs=qt_t[d0:d0 + 64, duo,
                                     qlo + poff + co:qlo + poff + co + n],
                            start=True,
                            stop=not (poff == 0 and co == 0),
                        )
                    if poff == 0:
                        nc.tensor.matmul(
                            sc[:, 0:128], lhsT=masku_t[:, 0:128],
                            rhs=ident_t[:], start=False, stop=True,
                            skip_group_check=True)
                    if kt >= 14:
                        dst = p_tail[(duo, hh, kt)][:, poff:poff + pw]
                    else:
                        p = kt // 2
                        par = kt % 2
                        off = 128 * par + poff
                        dst = p_pair[(duo, hh, p)][:, par, off:off + pw]
                    acc = (sums_t[:, hh * 2 + pi:hh * 2 + pi + 1]
                           if sums_t is not None else rs_t[:, hh:hh + 1])
                    nc.scalar.activation(
                        dst, sc[:, 0:pw], EXP,
                        bias=biasc_t[:, kt:kt + 1], scale=SCALE,
                        accum_out=acc)
            if sums_t is not None:
                for hh in range(2):
                    nc.vector.reduce_sum(
                        rs_t[:, hh:hh + 1], sums_t[:, hh * 2:hh * 2 + 2],
                        axis=AX)
            return rs_t

        def emit_vtilde(duo, kt, rs_t):
            # rinv = 1/r~ ; V~ = 32*v*rinv (fp8 pairs / fp16 tail)
            rinv_t = st.tile([128, 2], F32, tag="rinv", name=f"ri{duo}_{kt}")
            nc.vector.reciprocal(rinv_t[:], rs_t[:])
            for hh in range(2):
                if kt >= 14:
                    dst = vp_tail[(duo, kt)][:, 64 * hh:64 * hh + 64]
                else:
                    dst = vp_pair[(duo, kt // 2)][:, kt % 2,
                                                  64 * hh:64 * hh + 64]
                nc.vector.tensor_scalar_mul(
                    dst,
                    v_t[:, kt, duo * 128 + 64 * hh:duo * 128 + 64 * hh + 64],
                    rinv_t[:, hh:hh + 1])

        def emit_av_qtile(duo, j, ot_sb):
            # O[q, d] for q-tile j: fp8 DoubleRow, P~ stationary, V~ moving
            # with both heads side-by-side; fp16 solo for key-tiles 14, 15.
            # Out partitions = queries (always base 0).
            ob = pv_tile(f"av{duo}_{j}")
            obq = ob[:, 0:128]
            plast = min(j // 2, NPAIR - 1)
            for hh in range(2):
                for p in range(plast + 1):
                    c0 = 128 * j - 256 * p
                    nc.tensor.matmul(
                        obq[:, 64 * hh:64 * hh + 64],
                        lhsT=p_pair[(duo, hh, p)][:, :, c0:c0 + 128],
                        rhs=vp_pair[(duo, p)][:, :, 64 * hh:64 * hh + 64],
                        start=(p == 0),
                        stop=(j < 14 and p == plast),
                        perf_mode=DRM,
                        skip_group_check=True,
                    )
                for kt in (14, 15):
                    if kt > j:
                        continue
                    c0 = 128 * j - 128 * kt
                    nc.tensor.matmul(
                        obq[:, 64 * hh:64 * hh + 64],
                        lhsT=p_tail[(duo, hh, kt)][:, c0:c0 + 128],
                        rhs=vp_tail[(duo, kt)][:, 64 * hh:64 * hh + 64],
                        start=False,
                        stop=(kt == min(j, 15)),
                        skip_group_check=True,
                    )
            nc.vector.tensor_scalar_mul(
                ot_sb[:, 128 * j:128 * j + 128], obq, 1.0 / 32.0)
            nc.sync.dma_start(
                ot_d.ap()[128 * j:128 * j + 128,
                          duo * 128:duo * 128 + 128],
                ot_sb[:, 128 * j:128 * j + 128])

        # ---- schedule ----
        # duo0 fillers: own Q/K chunks just-in-time, duo1 Q/K chunks and V
        # tiles pushed toward big-kt iterations (wider ACT windows)
        qk_sched = {13: [(0, False, 2)], 12: [(0, True, 2)],
                    9: [(0, False, 1)], 8: [(0, True, 1)],
                    7: [(1, False, 3)], 6: [(1, True, 3)],
                    5: [(0, False, 0)], 4: [(0, True, 0)],
                    3: [(1, False, 2)], 2: [(1, True, 2)],
                    1: [(1, False, 1), (1, True, 1)],
                    0: [(1, False, 0), (1, True, 0)]}

        # head: 256-col sub-chunks so kt15/kt14 scores unblock earliest
        emit_qk_chunk(0, False, 3, off=256, n=256)
        emit_qk_chunk(0, True, 3, off=256, n=256)
        emit_qk_chunk(0, False, 3, off=0, n=256)
        emit_qk_chunk(0, True, 3, off=0, n=256)

        ot_sbs = [osb.tile([128, T], F32, tag=f"osb{d}", name=f"osb{d}")
                  for d in range(2)]
        # duo0's 16 A@V q-tiles, injected early into duo1's phase A
        av0_sched = {0: (0, 1, 2), 1: (3, 4, 5), 2: (6, 7, 8),
                     3: (9, 10, 11), 4: (12, 13), 5: (14, 15)}

        # duo0: key-tiles DESCENDING (small score tiles first -> early exp
        # start under partial DMA); duo1: ASCENDING so its own A@V q-tiles
        # interleave as soon as their key-pairs complete (short tail).
        rs_pend = {}
        for kt in range(NKT - 1, -1, -1):
            rs_t = emit_scores_exp(0, kt)
            rs_pend[kt] = rs_t
            for args in qk_sched.get(kt, ()):
                emit_qk_chunk(*args)
            jj = kt + 4
            if jj <= NKT - 1:
                emit_v_tile(jj)
                emit_vtilde(0, jj, rs_pend.pop(jj))
            if kt == 0:
                for jj in (3, 2, 1, 0):
                    emit_v_tile(jj)
                    emit_vtilde(0, jj, rs_pend.pop(jj))
        for kt in range(NKT):
            rs_t = emit_scores_exp(1, kt)
            emit_vtilde(1, kt, rs_t)
            for j in av0_sched.get(kt, ()):
                emit_av_qtile(0, j, ot_sbs[0])
            if kt % 2 == 1 and kt <= 13:
                emit_av_qtile(1, kt - 1, ot_sbs[1])
                emit_av_qtile(1, kt, ot_sbs[1])
            elif kt >= 14:
                emit_av_qtile(1, kt, ot_sbs[1])


def _get_module():
    if "nc" not in _CACHE:
        _CACHE["nc"] = _build_module()
    return _CACHE["nc"]


def _host_tables():
    k = np.arange(T)
    c = np.where(
        k < T - 256,
        2.0 ** np.round(0.5 * np.log2(1.031 * (T - k))),
        1.0)
    biasc = (-np.log(c)).reshape(NKT, 128).T.astype(np.float32)
    qi = np.arange(128)
    masku = np.where(qi[:, None] < qi[None, :], NEG, 0.0).astype(np_bf16)
    ident = np.eye(128, dtype=np.float32).astype(np_bf16)
    ones = np.ones((1, 128), np.float32).astype(np_bf16)
    return biasc, masku, ident, ones


def _split8(a):
    hi = a.astype(np_f8)
    lo = (a - hi.astype(np.float32)).astype(np_f8)
    return hi, lo


def _make_in_maps(X, Wq, bq, Wk, bk, Wv, bv):
    X = np.asarray(X, np.float32)
    biasc, masku, ident, ones = _host_tables()
    in_maps = []
    for core in range(8):
        b, g = divmod(core, 4)
        rows = slice(D2 * g, D2 * g + D2)
        xt = np.ascontiguousarray(X[b].T)              # [E, T]
        x8, xl = _split8(xt)
        x8 = np.ascontiguousarray(x8.reshape(8, 128, T).transpose(1, 0, 2))
        xl = np.ascontiguousarray(xl.reshape(8, 128, T).transpose(1, 0, 2))

        def wprep(Wfull):
            ws = np.asarray(Wfull)[rows].T.astype(np.float32) * WS  # [E, 256]
            return _split8(ws)

        wq8, wql = wprep(Wq)
        wk8, wkl = wprep(Wk)
        wv8, wvl = wprep(Wv)
        wqk8 = np.concatenate([wq8, wk8], axis=1)      # [E, 512]
        wqkl = np.concatenate([wql, wkl], axis=1)
        wv8l = np.concatenate([wv8, wvl], axis=1)

        def dr3(w):  # [E, 512] -> [128, 8, 512]
            return np.ascontiguousarray(
                w.reshape(8, 128, 512).transpose(1, 0, 2))

        bqc = (WS * np.asarray(bq)[rows]).reshape(2, 128).T
        bkc = (WS * np.asarray(bk)[rows]).reshape(2, 128).T
        cf = np.concatenate([biasc, bqc, bkc], axis=1).astype(np.float32)
        mi = np.concatenate([masku, ident], axis=1)
        ob = np.concatenate(
            [ones, (WS * np.asarray(bv)[rows]).reshape(1, D2).astype(np_bf16)],
            axis=1)
        in_maps.append({
            "x8": x8, "xl": xl,
            "wqk8": dr3(wqk8), "wqkl": dr3(wqkl), "wv8l": dr3(wv8l),
            "cf": np.ascontiguousarray(cf),
            "mi": np.ascontiguousarray(mi),
            "ob": np.ascontiguousarray(ob),
        })
    return in_maps


def kernel(X, Wq, bq, Wk, bk, Wv, bv, **kw):
    in_maps = _make_in_maps(X, Wq, bq, Wk, bk, Wv, bv)
    nc = _get_module()
    res = run_bass_kernel_spmd(nc, in_maps, core_ids=list(range(8)), **kw)
    _CACHE["last_res"] = res
    out = np.zeros((B, T, E), np.float32)
    for c in range(8):
        b, g = divmod(c, 4)
        out[b, :, D2 * g:D2 * g + D2] = res.results[c]["ot"]
    return out


if __name__ == "__main__":
    _get_module()
    print("module built ok")


# revision 23
# speedup vs baseline: 1.5316x; 1.0043x over previous
"""Multi-head attention (softmax over the QUERY axis) for Trainium2, 8 cores.

Reference (B=2, T=2048, E=1024, H=16, HD=64):
    q = X@Wq.T+bq ; k = ... ; v = ...   (per-head split)
    s = (q k^T)/sqrt(E), causal mask (key > query -> -inf)
    attn = softmax(s, axis=QUERY)  -> normalizes each key COLUMN over queries
    out  = attn @ v

Sharding: core c = (batch c//4, head-group c%4 of 4 heads = 2 duos).  No
collectives.

Math per core (d2=256 output dims as O^T [256, T]):
  Projections run in fp8e4 DoubleRow with hi+lo error compensation:
  W' = 32*W split W8+Wl (fp8), X split X8+Xl; q32 ~= X8@W8 + Xl@W8 + X8@Wl
  (+32b).  Scores use bf16 Q/K: s_psum = 1024*s; exp applies
  scale=1/32768 and per-key bias -ln(c_k), c_k ~ sqrt(E[r_k]) (any c_k is
  mathematically exact: V-scaling self-corrects).  P~=P/c_k stored fp8
  (keys < 1792) / fp16 (last 256 keys); V~ = 32*v*c/r.  A@V runs fp8
  DoubleRow over PAIRS of key-tiles (contraction 256); final PSUM->SBUF
  copy multiplies by 1/32.

Schedule: per duo, key-tiles processed DESCENDING (small score tiles
first -> exp starts ~7us in).  All P~/V~ live in SBUF; O^T accumulation
(phase B) runs bank-at-a-time overlapped with the next duo's phase A.
PSUM: 3x[128,1024] score slots + 2x[128,512] proj/V/O slots.
"""

import math
from contextlib import ExitStack

import numpy as np
import ml_dtypes

import concourse.bacc as bacc
import concourse.mybir as mybir
import concourse.tile as tile
from concourse.bass_utils import run_bass_kernel_spmd

B, T, E, H = 2, 2048, 1024, 16
D2 = 256              # output dims per core (4 heads)
NKT = 16              # key tiles of 128
NPAIR = 7             # fp8 DoubleRow pairs: kts (2p, 2p+1), p=0..6; kts 14,15 fp16
F32 = mybir.dt.float32
BF16 = mybir.dt.bfloat16
F16 = mybir.dt.float16
F8 = mybir.dt.float8e4
DRM = mybir.MatmulPerfMode.DoubleRow
EXP = mybir.ActivationFunctionType.Exp
AX = mybir.AxisListType.X
SCALE = 1.0 / 32768.0   # 1/sqrt(E) / 32 / 32  (both Q,K carry 32x weights)
NEG = -1.0e30
WS = 32.0               # weight prescale
np_f8 = ml_dtypes.float8_e4m3
np_bf16 = ml_dtypes.bfloat16

# w_t column-block offsets (per e-chunk, 1536 wide)
Q8O, K8O, QLO, KLO, V8O, VLO = 0, 256, 512, 768, 1024, 1280

_CACHE = {}


def _build_module():
    nc = bacc.Bacc("TRN2", target_bir_lowering=False, debug=False)

    x8_d = nc.dram_tensor("x8", [128, 8, T], F8, kind="ExternalInput")
    xl_d = nc.dram_tensor("xl", [128, 8, T], F8, kind="ExternalInput")
    wqk8_d = nc.dram_tensor("wqk8", [128, 8, 512], F8, kind="ExternalInput")
    wqkl_d = nc.dram_tensor("wqkl", [128, 8, 512], F8, kind="ExternalInput")
    wv8l_d = nc.dram_tensor("wv8l", [128, 8, 512], F8, kind="ExternalInput")
    # merged consts: fewer DMA instructions (each costs ~625ns of HWDGE)
    cf_d = nc.dram_tensor("cf", [128, NKT + 4], F32, kind="ExternalInput")
    mi_d = nc.dram_tensor("mi", [128, 256], BF16, kind="ExternalInput")
    ob_d = nc.dram_tensor("ob", [1, 128 + D2], BF16, kind="ExternalInput")
    ot_d = nc.dram_tensor("ot", [T, D2], F32, kind="ExternalOutput")

    with tile.TileContext(nc) as tc:
        _body(tc, x8_d, xl_d, wqk8_d, wqkl_d, wv8l_d, cf_d, mi_d, ob_d, ot_d)
    nc.compile()
    return nc


def _body(tc, x8_d, xl_d, wqk8_d, wqkl_d, wv8l_d, cf_d, mi_d, ob_d, ot_d):
    nc = tc.nc

    with ExitStack() as ctx:
        cp = ctx.enter_context(tc.tile_pool(name="const", bufs=1))
        cf_t = cp.tile([128, NKT + 4], F32)     # [biasc | bqc | bkc]
        mi_t = cp.tile([128, 256], BF16)        # [masku | ident]
        ob_t = cp.tile([1, 128 + D2], BF16)     # [ones | bvr]
        biasc_t = cf_t[:, 0:NKT]
        bqc_t = cf_t[:, NKT:NKT + 2]
        bkc_t = cf_t[:, NKT + 2:NKT + 4]
        masku_t = mi_t[:, 0:128]
        ident_t = mi_t[:, 128:256]
        ones_t = ob_t[:, 0:128]
        bvr_t = ob_t[:, 128:128 + D2]
        nc.sync.dma_start(ob_t[:], ob_d.ap())

        xw = ctx.enter_context(tc.tile_pool(name="xw", bufs=1))
        x8_t = xw.tile([128, 8, T], F8)
        xl_t = xw.tile([128, 8, T], F8)
        w_t = xw.tile([128, 8, 1536], F8)

        qk = ctx.enter_context(tc.tile_pool(name="qk", bufs=1))
        qt_t = qk.tile([128, 2, T], BF16)
        kt_t = qk.tile([128, 2, T], BF16)

        vtp = ctx.enter_context(tc.tile_pool(name="vt", bufs=1))
        v_t = vtp.tile([128, NKT, D2], BF16)

        pp = ctx.enter_context(tc.tile_pool(name="pp", bufs=1))
        vp = ctx.enter_context(tc.tile_pool(name="vp", bufs=1))
        st = ctx.enter_context(tc.tile_pool(name="st", bufs=6))
        osb = ctx.enter_context(tc.tile_pool(name="osb", bufs=1))

        sc_pool = ctx.enter_context(
            tc.tile_pool(name="scp", bufs=2, space="PSUM"))
        pv_pool = ctx.enter_context(
            tc.tile_pool(name="pvp", bufs=1, space="PSUM"))

        # ---- input DMA, ordered for earliest exp start ----
        nc.sync.dma_start(w_t[:, :, 0:512], wqk8_d.ap())
        nc.sync.dma_start(x8_t[:, :, 1536:2048], x8_d.ap()[:, :, 1536:2048])
        nc.sync.dma_start(w_t[:, :, 512:1024], wqkl_d.ap())
        nc.sync.dma_start(xl_t[:, :, 1536:2048], xl_d.ap()[:, :, 1536:2048])
        nc.sync.dma_start(mi_t[:], mi_d.ap())
        nc.sync.dma_start(cf_t[:], cf_d.ap())
        nc.sync.dma_start(w_t[:, :, 1024:1536], wv8l_d.ap())
        for tb in (2, 1, 0):
            nc.sync.dma_start(x8_t[:, :, tb * 512:tb * 512 + 512],
                              x8_d.ap()[:, :, tb * 512:tb * 512 + 512])
            nc.sync.dma_start(xl_t[:, :, tb * 512:tb * 512 + 512],
                              xl_d.ap()[:, :, tb * 512:tb * 512 + 512])

        # warm the exp table off the critical path
        warm_t = st.tile([1, 2], F32, name="warm")
        nc.scalar.activation(warm_t[:], ones_t[0:1, 0:2], EXP,
                             bias=0.0, scale=SCALE)

        # ---- P~ / V~ tiles (SBUF-resident until the duo's A@V) ----
        p_pair = {}   # (duo, hh, p) -> [128, 2, Wp] fp8
        p_tail = {}   # (duo, hh, kt in 14,15) -> [128, W] fp16
        vp_pair = {}  # (duo, p) -> [128, 2, 128] fp8   (both heads in free)
        vp_tail = {}  # (duo, kt) -> [128, 128] fp16
        for d in range(2):
            for hh in range(2):
                for p in range(NPAIR):
                    wp = T - 256 * p
                    p_pair[(d, hh, p)] = pp.tile(
                        [128, 2, wp], F8, tag=f"p{d}_{hh}_{p}",
                        name=f"p{d}_{hh}_{p}")
                for kt in (14, 15):
                    p_tail[(d, hh, kt)] = pp.tile(
                        [128, T - 128 * kt], F16, tag=f"pt{d}_{hh}_{kt}",
                        name=f"pt{d}_{hh}_{kt}")
            for kt in (14, 15):
                vp_tail[(d, kt)] = vp.tile(
                    [128, 128], F16, tag=f"vpt{d}_{kt}",
                    name=f"vpt{d}_{kt}")
            for p in range(NPAIR):
                vp_pair[(d, p)] = vp.tile(
                    [128, 2, 128], F8, tag=f"vp{d}_{p}",
                    name=f"vp{d}_{p}")
        # zero the odd-kt first-128 strips (masked region the exp never
        # writes); gpsimd keeps this off the busy engines
        for d in range(2):
            for hh in range(2):
                for p in range(NPAIR):
                    nc.gpsimd.memset(p_pair[(d, hh, p)][:, 1, 0:128], 0.0)

        # ---- PE helper emitters ----
        pv_tog = [0]

        def pv_tile(name):
            tag = ("pj", "ob")[pv_tog[0] % 2]
            pv_tog[0] += 1
            return pv_pool.tile([128, 512], F32, tag=tag, name=name)

        def emit_qk_chunk(duo, is_k, c, off=0, n=512):
            # one chunk of the Q^T/K^T projection for `duo`:
            # psum = X8.T@W8 + Xl.T@W8 + X8.T@Wl  (12 fp8 DoubleRow matmuls)
            w8o, wlo = (K8O, KLO) if is_k else (Q8O, QLO)
            out_t, b_t = (kt_t, bkc_t) if is_k else (qt_t, bqc_t)
            c0 = c * 512 + off
            ps = pv_tile(f"qk{duo}_{int(is_k)}_{c}_{off}")
            first = True
            for xs, wo in ((x8_t, w8o), (x8_t, wlo), (xl_t, w8o)):
                for ep in range(4):
                    nc.tensor.matmul(
                        ps[:, 0:n],
                        lhsT=w_t[:, 2 * ep:2 * ep + 2,
                                 wo + duo * 128:wo + duo * 128 + 128],
                        rhs=xs[:, 2 * ep:2 * ep + 2, c0:c0 + n],
                        start=first,
                        stop=(xs is xl_t and ep == 3),
                        perf_mode=DRM,
                    )
                    first = False
            nc.vector.tensor_scalar_add(
                out_t[:, duo, c0:c0 + n], ps[:, 0:n],
                b_t[:, duo:duo + 1])

        def emit_v_tile(kt):
            # V tile (both duos): [128 t, 256 d] = X.T@Wv*32 + 32*bv
            ps = pv_tile(f"v{kt}")
            pvs = ps[:, 0:D2]
            for xs, wo in ((x8_t, V8O), (xl_t, V8O), (x8_t, VLO)):
                for ep in range(4):
                    nc.tensor.matmul(
                        pvs,
                        lhsT=xs[:, 2 * ep:2 * ep + 2,
                                kt * 128:kt * 128 + 128],
                        rhs=w_t[:, 2 * ep:2 * ep + 2, wo:wo + D2],
                        start=(xs is x8_t and wo == V8O and ep == 0),
                        stop=False,
                        perf_mode=DRM,
                    )
            nc.tensor.matmul(pvs, lhsT=ones_t[0:1, :], rhs=bvr_t[0:1, :],
                             start=False, stop=True)
            nc.vector.tensor_copy(v_t[:, kt, :], pvs)

        def emit_scores_exp(duo, kt):
            # scores S^T[key, q] for q in [qlo, T), exp'd into P~ with
            # per-key bias -ln(c_k); accum -> rs (per-key sums r~)
            qlo = kt * 128
            w = T - qlo
            pieces = [(0, min(w, 1536))]
            if w > 1536:
                pieces.append((1536, w - 1536))
            rs_t = st.tile([128, 2], F32, tag="rs", name=f"rs{duo}_{kt}")
            sums_t = (st.tile([128, 4], F32, tag="sums", name=f"sm{duo}_{kt}")
                      if len(pieces) > 1 else None)
            for hh in range(2):
                d0 = 64 * hh
                for pi, (poff, pw) in enumerate(pieces):
                    sc = sc_pool.tile([128, 1536], F32, tag="sc", name="sc")
                    for co in range(0, pw, 512):
                        n = min(512, pw - co)
                        nc.tensor.matmul(
                            sc[:, co:co + n],
                            lhsT=kt_t[d0:d0 + 64, duo, qlo:qlo + 128],
                            rhs=qt_t[d0:d0 + 64, duo,
                                     qlo + poff + co:qlo + poff + co + n],
                            start=True,
                            stop=not (poff == 0 and co == 0),
                        )
                    if poff == 0:
                        nc.tensor.matmul(
                            sc[:, 0:128], lhsT=masku_t[:, 0:128],
                            rhs=ident_t[:], start=False, stop=True,
                            skip_group_check=True)
                    if kt >= 14:
                        dst = p_tail[(duo, hh, kt)][:, poff:poff + pw]
                    else:
                        p = kt // 2
                        par = kt % 2
                        off = 128 * par + poff
                        dst = p_pair[(duo, hh, p)][:, par, off:off + pw]
                    acc = (sums_t[:, hh * 2 + pi:hh * 2 + pi + 1]
                           if sums_t is not None else rs_t[:, hh:hh + 1])
                    nc.scalar.activation(
                        dst, sc[:, 0:pw], EXP,
                        bias=biasc_t[:, kt:kt + 1], scale=SCALE,
                        accum_out=acc)
            if sums_t is not None:
                for hh in range(2):
                    nc.vector.reduce_sum(
                        rs_t[:, hh:hh + 1], sums_t[:, hh * 2:hh * 2 + 2],
                        axis=AX)
            return rs_t

        def emit_vtilde(duo, kt, rs_t):
            # rinv = 1/r~ ; V~ = 32*v*rinv (fp8 pairs / fp16 tail)
            rinv_t = st.tile([128, 2], F32, tag="rinv", name=f"ri{duo}_{kt}")
            nc.vector.reciprocal(rinv_t[:], rs_t[:])
            for hh in range(2):
                if kt >= 14:
                    dst = vp_tail[(duo, kt)][:, 64 * hh:64 * hh + 64]
                else:
                    dst = vp_pair[(duo, kt // 2)][:, kt % 2,
                                                  64 * hh:64 * hh + 64]
                nc.vector.tensor_scalar_mul(
                    dst,
                    v_t[:, kt, duo * 128 + 64 * hh:duo * 128 + 64 * hh + 64],
                    rinv_t[:, hh:hh + 1])

        def emit_av_qtile(duo, j, ot_sb):
            # O[q, d] for q-tile j: fp8 DoubleRow, P~ stationary, V~ moving
            # with both heads side-by-side; fp16 solo for key-tiles 14, 15.
            # Out partitions = queries (always base 0).
            ob = pv_tile(f"av{duo}_{j}")
            obq = ob[:, 0:128]
            plast = min(j // 2, NPAIR - 1)
            for hh in range(2):
                for p in range(plast + 1):
                    c0 = 128 * j - 256 * p
                    nc.tensor.matmul(
                        obq[:, 64 * hh:64 * hh + 64],
                        lhsT=p_pair[(duo, hh, p)][:, :, c0:c0 + 128],
                        rhs=vp_pair[(duo, p)][:, :, 64 * hh:64 * hh + 64],
                        start=(p == 0),
                        stop=(j < 14 and p == plast),
                        perf_mode=DRM,
                        skip_group_check=True,
                    )
                for kt in (14, 15):
                    if kt > j:
                        continue
                    c0 = 128 * j - 128 * kt
                    nc.tensor.matmul(
                        obq[:, 64 * hh:64 * hh + 64],
                        lhsT=p_tail[(duo, hh, kt)][:, c0:c0 + 128],
                        rhs=vp_tail[(duo, kt)][:, 64 * hh:64 * hh + 64],
                        start=False,
                        stop=(kt == min(j, 15)),
                        skip_group_check=True,
                    )
            nc.vector.tensor_scalar_mul(
                ot_sb[:, 128 * j:128 * j + 128], obq, 1.0 / 32.0)
            nc.sync.dma_start(
                ot_d.ap()[128 * j:128 * j + 128,
                          duo * 128:duo * 128 + 128],
                ot_sb[:, 128 * j:128 * j + 128])

        # ---- schedule ----
        # duo0 fillers: own Q/K chunks just-in-time, duo1 Q/K chunks and V
        # tiles pushed toward big-kt iterations (wider ACT windows)
        qk_sched = {13: [(0, False, 2)], 12: [(0, True, 2)],
                    9: [(0, False, 1)], 8: [(0, True, 1)],
                    7: [(1, False, 3)], 6: [(1, True, 3)],
                    5: [(0, False, 0)], 4: [(0, True, 0)],
                    3: [(1, False, 2)], 2: [(1, True, 2)],
                    1: [(1, False, 1), (1, True, 1)],
                    0: [(1, False, 0), (1, True, 0)]}

        # head: 256-col sub-chunks; the [1792:2048] halves unblock kt15/14,
        # the [1536:1792] halves are emitted BETWEEN kt15 and kt13 (PE runs
        # its queue in program order)
        emit_qk_chunk(0, False, 3, off=256, n=256)
        emit_qk_chunk(0, True, 3, off=256, n=256)
        head_sched = {15: [(0, False, 3, 0, 256)],
                      14: [(0, True, 3, 0, 256)]}
        qk_sched = {k: head_sched.get(k, []) + qk_sched.get(k, [])
                    for k in set(head_sched) | set(qk_sched)}

        ot_sbs = [osb.tile([128, T], F32, tag=f"osb{d}", name=f"osb{d}")
                  for d in range(2)]
        # duo0's 16 A@V q-tiles, injected early into duo1's phase A
        av0_sched = {0: (0, 1, 2), 1: (3, 4, 5), 2: (6, 7, 8),
                     3: (9, 10, 11), 4: (12, 13), 5: (14, 15)}

        # duo0: key-tiles DESCENDING (small score tiles first -> early exp
        # start under partial DMA); duo1: ASCENDING so its own A@V q-tiles
        # interleave as soon as their key-pairs complete (short tail).
        rs_pend = {}
        for kt in range(NKT - 1, -1, -1):
            rs_t = emit_scores_exp(0, kt)
            rs_pend[kt] = rs_t
            for args in qk_sched.get(kt, ()):
                emit_qk_chunk(*args)
            jj = kt + 4
            if jj <= NKT - 1:
                emit_v_tile(jj)
                emit_vtilde(0, jj, rs_pend.pop(jj))
            if kt == 0:
                for jj in (3, 2, 1, 0):
                    emit_v_tile(jj)
                    emit_vtilde(0, jj, rs_pend.pop(jj))
        for kt in range(NKT):
            rs_t = emit_scores_exp(1, kt)
            emit_vtilde(1, kt, rs_t)
            for j in av0_sched.get(kt, ()):
                emit_av_qtile(0, j, ot_sbs[0])
            if kt % 2 == 1 and kt <= 13:
                emit_av_qtile(1, kt - 1, ot_sbs[1])
                emit_av_qtile(1, kt, ot_sbs[1])
            elif kt >= 14:
                emit_av_qtile(1, kt, ot_sbs[1])


def _get_module():
    if "nc" not in _CACHE:
        _CACHE["nc"] = _build_module()
    return _CACHE["nc"]


def _host_tables():
    k = np.arange(T)
    c = np.where(
        k < T - 256,
        2.0 ** np.round(0.5 * np.log2(1.031 * (T - k))),
        1.0)
    biasc = (-np.log(c)).reshape(NKT, 128).T.astype(np.float32)
    qi = np.arange(128)
    masku = np.where(qi[:, None] < qi[None, :], NEG, 0.0).astype(np_bf16)
    ident = np.eye(128, dtype=np.float32).astype(np_bf16)
    ones = np.ones((1, 128), np.float32).astype(np_bf16)
    return biasc, masku, ident, ones


def _split8(a):
    hi = a.astype(np_f8)
    lo = (a - hi.astype(np.float32)).astype(np_f8)
    return hi, lo


def _make_in_maps(X, Wq, bq, Wk, bk, Wv, bv):
    X = np.asarray(X, np.float32)
    biasc, masku, ident, ones = _host_tables()
    in_maps = []
    for core in range(8):
        b, g = divmod(core, 4)
        rows = slice(D2 * g, D2 * g + D2)
        xt = np.ascontiguousarray(X[b].T)              # [E, T]
        x8, xl = _split8(xt)
        x8 = np.ascontiguousarray(x8.reshape(8, 128, T).transpose(1, 0, 2))
        xl = np.ascontiguousarray(xl.reshape(8, 128, T).transpose(1, 0, 2))

        def wprep(Wfull):
            ws = np.asarray(Wfull)[rows].T.astype(np.float32) * WS  # [E, 256]
            return _split8(ws)

        wq8, wql = wprep(Wq)
        wk8, wkl = wprep(Wk)
        wv8, wvl = wprep(Wv)
        wqk8 = np.concatenate([wq8, wk8], axis=1)      # [E, 512]
        wqkl = np.concatenate([wql, wkl], axis=1)
        wv8l = np.concatenate([wv8, wvl], axis=1)

        def dr3(w):  # [E, 512] -> [128, 8, 512]
            return np.ascontiguousarray(
                w.reshape(8, 128, 512).transpose(1, 0, 2))

        bqc = (WS * np.asarray(bq)[rows]).reshape(2, 128).T
        bkc = (WS * np.asarray(bk)[rows]).reshape(2, 128).T
        cf = np.concatenate([biasc, bqc, bkc], axis=1).astype(np.float32)
        mi = np.concatenate([masku, ident], axis=1)
        ob = np.concatenate(
            [ones, (WS * np.asarray(bv)[rows]).reshape(1, D2).astype(np_bf16)],
            axis=1)
        in_maps.append({
            "x8": x8, "xl": xl,
            "wqk8": dr3(wqk8), "wqkl": dr3(wqkl), "wv8l": dr3(wv8l),
            "cf": np.ascontiguousarray(cf),
            "mi": np.ascontiguousarray(mi),
            "ob": np.ascontiguousarray(ob),
        })
    return in_maps


def kernel(X, Wq, bq, Wk, bk, Wv, bv, **kw):
    in_maps = _make_in_maps(X, Wq, bq, Wk, bk, Wv, bv)
    nc = _get_module()
    res = run_bass_kernel_spmd(nc, in_maps, core_ids=list(range(8)), **kw)
    _CACHE["last_res"] = res
    out = np.zeros((B, T, E), np.float32)
    for c in range(8):
        b, g = divmod(c, 4)
        out[b, :, D2 * g:D2 * g + D2] = res.results[c]["ot"]
    return out


if __name__ == "__main__":
    _get_module()
    print("module built ok")
